# revision 1
# baseline (speedup 1.0000x reference)
"""EGNN (N=384, D=3, H=128, L=4) Bass kernel for 8 TRN2 NeuronCores.

Sharding: rows of the N x N edge grid split across 8 cores (48 rows each).
Each core holds full h; per layer it computes its row-block of the coord/edge
MLPs and row-sums (phi-weighted coordinate update, gated message sum), then
AllGathers the per-node x updates and msum rows. The h-node MLP is computed
redundantly on every core (384 cols, tiny). The embedding row-sum of
emb_w (49152 x 384, the dominant HBM traffic) is sharded 1/8 per core.

Self-contained: shapes hardcoded, inputs are the full unsharded arrays.
"""
import os
import numpy as np

DEBUG = os.environ.get("KDEBUG", "0") == "1"
BF16MLP = os.environ.get("KBF16", "0") == "1"

N, D, H, L = 384, 3, 128, 4
NC = 8
NI = N // NC          # 48 rows per core
NJ = N                # 384 cols
G = 2                 # i-rows per group
NGRP = NI // G        # 24 groups
EMB_ROWS = N * H // NC  # 6144 emb_w rows per core

_cache = {}


def _build_nc():
    import concourse.bass as bass
    import concourse.bacc as bacc
    import concourse.tile as tile
    from concourse import mybir

    F32 = mybir.dt.float32
    AF = mybir.ActivationFunctionType
    OP = mybir.AluOpType

    nc = bacc.Bacc(None, target_bir_lowering=False)
    F32R = mybir.dt.float32r

    def RMM(out, lhsT, rhs, **kw):
        nc.tensor.matmul(out, lhsT=lhsT.bitcast(F32R), rhs=rhs.bitcast(F32R), **kw)

    def P(name, shape):
        return nc.declare_dram_parameter(name, list(shape), F32, isOutput=False)

    # per-core inputs
    embw = P("embw", (EMB_ROWS, NJ))
    embbT = P("embbT", (H, NI))
    x0my = P("x0my", (NI, D))
    maskc = P("maskc", (NI, NJ))
    eyec = P("eyec", (NI, NJ))
    # shared inputs
    x0rows = P("x0rows", (1, D * NJ))
    c1hiT = P("c1hiT", (L, H, H))
    c1hjT = P("c1hjT", (L, H, H))
    c1drep = P("c1drep", (L, 16 * H))
    cb1 = P("cb1", (H, L))
    c2T = P("c2T", (L, H, H))
    cb2 = P("cb2", (H, L))
    c3w = P("c3w", (L, H, 2 * NI - 1))
    cb3c = P("cb3c", (NI, L))
    e1hiT = P("e1hiT", (L - 1, H, H))
    e1hjT = P("e1hjT", (L - 1, H, H))
    e1drep = P("e1drep", (L - 1, 16 * H))
    ones6k = P("ones6k", (1, 16 * NJ))
    eb1 = P("eb1", (H, L - 1))
    e2T = P("e2T", (L - 1, H, H))
    eb2 = P("eb2", (H, L - 1))
    attw = P("attw", (L - 1, H, 2 * NI - 1))
    nw1hT = P("nw1hT", (L - 1, H, H))
    nw1mT = P("nw1mT", (L - 1, H, H))
    nb1 = P("nb1", (H, L - 1))
    nw2T = P("nw2T", (L - 1, H, H))
    nb2 = P("nb2", (H, L - 1))
    ones128 = P("ones128", (1, H))

    o_x = nc.declare_dram_parameter("o_x", [N, D], F32, isOutput=True)
    dbg = {}
    if DEBUG:
        for nm, shp in [("h0", (H, NJ)), ("d2", (NI, NJ)), ("u", (NI, NJ)),
                        ("t1c", (H, G * NJ)), ("phis", (NI, NJ)),
                        ("msum", (H, NI)), ("x1", (NI, D)), ("h1", (H, NJ)),
                        ("gmask", (NI, NJ))]:
            dbg[nm] = nc.declare_dram_parameter("dbg_" + nm, list(shp), F32,
                                                isOutput=True)

    # collective bounce buffers
    hag_in = nc.dram_tensor("hag_in", [H, NI], F32)
    hag_out = nc.dram_tensor("hag_out", [NC * H, NI], F32, addr_space="Shared")
    xag_in = [nc.dram_tensor(f"xag_in{l}", [D, NI], F32) for l in range(L - 1)]
    xag_in.append(nc.dram_tensor(f"xag_in3", [NI, D], F32))
    xag_out = [nc.dram_tensor(f"xag_out{l}", [NC, D, NI], F32, addr_space="Shared")
               for l in range(L - 1)]
    xag_out.append(nc.dram_tensor(f"xag_out3", [N, D], F32, addr_space="Shared"))
    mag_in = [nc.dram_tensor(f"mag_in{l}", [H, NI], F32) for l in range(L - 1)]
    mag_out = [nc.dram_tensor(f"mag_out{l}", [NC * H, NI], F32, addr_space="Shared")
               for l in range(L - 1)]
    rg = [list(range(NC))]

    with tile.TileContext(nc) as tc:
        with (
            tc.tile_pool(name="consts", bufs=1) as consts,
            tc.tile_pool(name="embp", bufs=2) as embp,
            tc.tile_pool(name="work", bufs=2) as work,
            tc.tile_pool(name="slab", bufs=1) as slabp,
            tc.tile_pool(name="cp1", bufs=1) as cp1,
            tc.tile_pool(name="cp2", bufs=2) as cp2,
            tc.tile_pool(name="rows", bufs=2) as rowsp,
            tc.tile_pool(name="mgp", bufs=1) as mgp,
            tc.tile_pool(name="ps_mlp", bufs=3, space="PSUM") as ps_mlp,
            tc.tile_pool(name="ps_sm", bufs=2, space="PSUM") as ps_sm,
        ):
            # ---- load constants ----
            def load(pname, ap_in, shape, rnd=False):
                t = consts.tile(list(shape), F32, tag=pname)
                o = t[:].bitcast(F32R) if rnd else t
                nc.sync.dma_start(out=o, in_=ap_in.bitcast(F32R) if rnd else ap_in)
                return t

            c1hiT_sb = load("c1hiT", c1hiT.rearrange("l p x -> p l x"), (H, L, H),
                            rnd=True)
            c1hjT_sb = load("c1hjT", c1hjT.rearrange("l p x -> p l x"), (H, L, H),
                            rnd=True)
            c2T_sb = load("c2T", c2T.rearrange("l p x -> p l x"), (H, L, H),
                          rnd=True)
            c3w_sb = load("c3w", c3w.rearrange("l p x -> p l x"),
                          (H, L, 2 * NI - 1), rnd=True)

            cb1_sb = load("cb1", cb1[:], (H, L))
            cb2_sb = load("cb2", cb2[:], (H, L))
            cb3c_sb = load("cb3c", cb3c[:], (NI, L))
            e1hiT_sb = load("e1hiT", e1hiT.rearrange("l p x -> p l x"),
                            (H, L - 1, H), rnd=True)
            e1hjT_sb = load("e1hjT", e1hjT.rearrange("l p x -> p l x"),
                            (H, L - 1, H), rnd=True)
            e2T_sb = load("e2T", e2T.rearrange("l p x -> p l x"), (H, L - 1, H),
                          rnd=True)
            attw_sb = load("attw", attw.rearrange("l p x -> p l x"),
                           (H, L - 1, 2 * NI - 1), rnd=True)

            eb1_sb = load("eb1", eb1[:], (H, L - 1))
            eb2_sb = load("eb2", eb2[:], (H, L - 1))
            nw1hT_sb = load("nw1hT", nw1hT.rearrange("l p x -> p l x"),
                            (H, L - 1, H), rnd=True)
            nw1mT_sb = load("nw1mT", nw1mT.rearrange("l p x -> p l x"),
                            (H, L - 1, H), rnd=True)
            nw2T_sb = load("nw2T", nw2T.rearrange("l p x -> p l x"),
                           (H, L - 1, H), rnd=True)
            nb1_sb = load("nb1", nb1[:], (H, L - 1))
            nb2_sb = load("nb2", nb2[:], (H, L - 1))
            maskc_sb = load("maskc", maskc[:], (NI, NJ))
            eyec_sb = load("eyec", eyec[:], (NI, NJ))
            embbT_sb = load("embbT", embbT[:], (H, NI))
            ones_sb = load("ones128", ones128[:], (1, H), rnd=True)
            x0my_sb = load("x0my", x0my[:], (NI, D))

            BF16 = mybir.dt.bfloat16
            MLPDT = BF16 if BF16MLP else F32
            def MLPV(ap):
                # view for matmul operands of MLP-pass tiles
                return ap if BF16MLP else ap.bitcast(F32R)

            if BF16MLP:
                def tobf(t, pname):
                    b = consts.tile(list(t.shape), BF16, tag=pname + "_bf")
                    nc.vector.tensor_copy(b, t)
                    return b
                c1hjT_w = tobf(c1hjT_sb, "c1hjT")
                c2T_w = tobf(c2T_sb, "c2T")
                c3w_w = tobf(c3w_sb, "c3w")
                e1hjT_w = tobf(e1hjT_sb, "e1hjT")
                e2T_w = tobf(e2T_sb, "e2T")
                attw_w = tobf(attw_sb, "attw")
                ones_w = tobf(ones_sb, "ones128")
            else:
                c1hjT_w, c2T_w = c1hjT_sb, c2T_sb
                c3w_w, e1hjT_w = c3w_sb, e1hjT_sb
                e2T_w, attw_w, ones_w = e2T_sb, attw_sb, ones_sb

            d2ones = consts.tile([66, 16 * NJ], F32, tag="d2ones")
            combC = consts.tile([66, 16 * H], F32, tag="combC")
            combE = consts.tile([66, 16 * H], F32, tag="combE")
            for k in range(3):
                nc.sync.dma_start(out=d2ones[32 * k + 1:32 * k + 2, :]
                                  .bitcast(F32R), in_=ones6k[:].bitcast(F32R))

            # ---- phase 0: embedding row-sum (memory bound) ----
            hT0 = cp1.tile([H, NI], F32, tag="hT0")
            EB = 4  # nodes per embedding DMA
            for t in range(NI // EB):
                et = embp.tile([H, EB, NJ], F32, tag="embt")
                eng = nc.sync if t % 2 == 0 else nc.scalar
                eng.dma_start(
                    out=et,
                    in_=embw[t * EB * H:(t + 1) * EB * H, :]
                    .rearrange("(a p) j -> p a j", p=H))
                with nc.allow_low_precision(reason="f32r storage, f32 accum"):
                    nc.vector.tensor_reduce(
                        out=hT0[:, t * EB:(t + 1) * EB].bitcast(F32R), in_=et,
                        axis=mybir.AxisListType.X, op=OP.add,
                    )
            nc.vector.tensor_tensor(out=hT0[:].bitcast(F32R), in0=hT0,
                                    in1=embbT_sb, op=OP.add)
            nc.sync.dma_start(out=hag_in[:], in_=hT0)
            nc.gpsimd.collective_compute(
                "AllGather", OP.bypass, replica_groups=rg,
                ins=[hag_in[:]], outs=[hag_out[:]],
            )
            h_T = cp2.tile([H, NJ], F32, tag="hT")
            for r in range(NC):
                nc.sync.dma_start(out=h_T[:, r * NI:(r + 1) * NI].bitcast(F32R),
                                  in_=hag_out[r * H:(r + 1) * H, :].bitcast(F32R))
            h_my = hT0  # this core's own h rows (h_T columns 48c..48c+48)
            if DEBUG:
                nc.sync.dma_start(out=dbg["h0"][:], in_=h_T)

            x_my = x0my_sb

            for l in range(L):
                last = l == L - 1
                # ---- A: coordinate prep ----
                diff = []
                for c in range(D):
                    xb = cp1.tile([NI, NJ], F32, tag=f"xb{c}")
                    if l == 0:
                        bsrc = bass.AP(tensor=x0rows, offset=c * NJ,
                                       ap=[[0, NI], [1, NJ]])
                    else:
                        bsrc = bass.AP(tensor=xag_out[l - 1], offset=c * NI,
                                       ap=[[0, NI], [D * NI, NC], [1, NI]])
                    nc.sync.dma_start(out=xb, in_=bsrc)
                    dc = cp2.tile([NI, NJ], F32, tag=f"diff{c}")
                    nc.vector.tensor_scalar(
                        out=dc, in0=xb, scalar1=x_my[:, c:c + 1], scalar2=None,
                        op0=OP.subtract,
                    )
                    diff.append(dc)
                d2 = cp1.tile([NI, NJ], F32, tag="d2")
                tmp = cp1.tile([NI, NJ], F32, tag="ctmp")
                nc.vector.tensor_tensor(out=d2, in0=diff[0], in1=diff[0], op=OP.mult)
                nc.vector.tensor_tensor(out=tmp, in0=diff[1], in1=diff[1], op=OP.mult)
                nc.vector.tensor_tensor(out=d2, in0=d2, in1=tmp, op=OP.add)
                nc.vector.tensor_tensor(out=tmp, in0=diff[2], in1=diff[2], op=OP.mult)
                nc.vector.tensor_tensor(out=d2, in0=d2, in1=tmp, op=OP.add)
                d2s = cp1.tile([NI, NJ], F32, tag="d2s")
                nc.vector.tensor_tensor(out=d2s, in0=d2, in1=eyec_sb, op=OP.add)
                dn = cp1.tile([NI, NJ], F32, tag="dn")
                nc.scalar.activation(out=dn, in_=d2s, func=AF.Sqrt)
                nc.vector.tensor_scalar(out=dn, in0=dn, scalar1=1.0, scalar2=None,
                                        op0=OP.add)
                u = cp2.tile([NI, NJ], F32, tag="u")
                nc.vector.reciprocal(out=u, in_=dn)

                if DEBUG and l == 0:
                    nc.sync.dma_start(out=dbg["d2"][:], in_=d2)
                    nc.sync.dma_start(out=dbg["u"][:], in_=u)

                # ---- B: edge-grid MLP groups ----
                hT_l = h_T    # h for THIS layer (h_T gets rebound by node MLP)
                hmy_l = h_my
                if BF16MLP:
                    hT_mlp = cp2.tile([H, NJ], BF16, tag="hTb")
                    nc.vector.tensor_copy(hT_mlp, h_T)
                    d2_mlp = cp1.tile([NI, NJ], BF16, tag="d2b")
                    nc.vector.tensor_copy(d2_mlp, d2)
                else:
                    hT_mlp = h_T
                    d2_mlp = d2
                for k in range(3):
                    nc.sync.dma_start(
                        out=d2ones[32 * k:32 * k + 1, :].bitcast(F32R),
                        in_=d2_mlp[16 * k:16 * (k + 1), :].bitcast(F32R))

                def make_comb(hiT, wdrep_l, comb, atag):
                    # A_T = (W1hi @ h_my).T via one matmul; stripe [wd; A_row]
                    # pairs into comb for the K=2 d-pass
                    At_ps = ps_sm.tile([NI, H], F32, tag="sm")
                    RMM(At_ps, lhsT=hmy_l, rhs=hiT, start=True, stop=True)
                    At = cp1.tile([NI, H], F32, tag=atag)
                    nc.vector.tensor_copy(At[:].bitcast(F32R), At_ps)
                    for k in range(3):
                        nc.sync.dma_start(
                            out=comb[32 * k:32 * k + 1, :].bitcast(F32R),
                            in_=wdrep_l.bitcast(F32R))
                        nc.sync.dma_start(
                            out=comb[32 * k + 1:32 * k + 2, :].bitcast(F32R),
                            in_=At[16 * k:16 * (k + 1), :].bitcast(F32R))

                make_comb(c1hiT_sb[:, l, :], c1drep[l:l + 1, :], combC, "Atc")
                if not last:
                    make_comb(e1hiT_sb[:, l, :], e1drep[l:l + 1, :], combE, "Ate")

                def emit_group(comb, b1c, hjT, w2T, b2c, winT, acc_ps,
                               slab, g):
                    pre = ps_mlp.tile([H, G * 512], F32, tag="mlp")

                    def mm_hj(first):
                        for r in range(G):
                            nc.tensor.matmul(
                                pre[:, r * 512:r * 512 + NJ],
                                lhsT=MLPV(hjT),
                                rhs=MLPV(hT_mlp[:, :]),
                                start=first, stop=not first)

                    def mm_d(first):
                        for r in range(G):
                            i = G * g + r
                            kq, iq = i // 16, i % 16
                            nc.tensor.matmul(
                                pre[:, r * 512:r * 512 + NJ],
                                lhsT=comb[32 * kq:32 * kq + 2,
                                          iq * H:(iq + 1) * H].bitcast(F32R),
                                rhs=d2ones[32 * kq:32 * kq + 2,
                                           iq * NJ:(iq + 1) * NJ].bitcast(F32R),
                                start=first, stop=not first)

                    mm_hj(True)
                    mm_d(False)
                    t1 = work.tile([H, G * NJ], MLPDT, tag="t1")
                    nc.scalar.activation(
                        out=(t1[:, :] if BF16MLP else t1[:, :].bitcast(F32R))
                        .rearrange("p (r c) -> p r c", r=G),
                        in_=pre.rearrange("p (r c) -> p r c", r=G)[:, :, 0:NJ],
                        func=AF.Silu, bias=b1c, scale=1.0,
                    )
                    z2 = ps_mlp.tile([H, G * 512], F32, tag="mlp")
                    nc.tensor.matmul(z2[:, 0:512], lhsT=MLPV(w2T),
                                     rhs=MLPV(t1[:, 0:512]),
                                     start=True, stop=True)
                    nc.tensor.matmul(z2[:, 512:768], lhsT=MLPV(w2T),
                                     rhs=MLPV(t1[:, 512:768]),
                                     start=True, stop=True)
                    if slab is not None:
                        t2 = slab[:, g * (G * NJ):(g + 1) * (G * NJ)]
                    else:
                        t2 = work.tile([H, G * NJ], MLPDT, tag="t2")
                    nc.scalar.activation(
                        out=t2 if BF16MLP else t2.bitcast(F32R),
                        in_=z2[:, 0:G * NJ],
                        func=AF.Silu, bias=b2c, scale=1.0,
                    )
                    if acc_ps is not None:
                        pend.append((t2, g))
                        if len(pend) > 1:
                            flush_acc(pend.pop(0), acc_ps, winT)

                pend = []

                def flush_acc(item, acc_ps, winT):
                    t2p, gp = item
                    for r in range(G):
                        i = G * gp + r
                        nc.tensor.matmul(
                            acc_ps,
                            lhsT=MLPV(winT[:, (NI - 1) - i:(2 * NI - 1) - i]),
                            rhs=MLPV(t2p[:, r * NJ:(r + 1) * NJ]),
                            start=(i == 0), stop=(i == NI - 1),
                        )

                phi_ps = ps_sm.tile([H, NJ], F32, tag="sm")

                def coord_group(g):
                    emit_group(combC, cb1_sb[:, l:l + 1], c1hjT_w[:, l, :],
                               c2T_w[:, l, :], cb2_sb[:, l:l + 1],
                               c3w_w[:, l, :], phi_ps[0:NI, :], None, g)

                if not last:
                    att_ps = ps_sm.tile([H, NJ], F32, tag="sm")
                    m2slab = slabp.tile([H, NI * NJ], MLPDT, tag="m2")
                    # edge groups, with early coord groups stitched in to keep
                    # the PE fed during edge-phase ACT waits
                    for g in range(NGRP):
                        emit_group(combE, eb1_sb[:, l:l + 1], e1hjT_w[:, l, :],
                                   e2T_w[:, l, :], eb2_sb[:, l:l + 1],
                                   attw_w[:, l, :], None, m2slab, g)
                    for i in range(NI):
                        nc.tensor.matmul(
                            att_ps[0:NI, :],
                            lhsT=MLPV(attw_w[:, l, (NI - 1) - i:(2 * NI - 1) - i]),
                            rhs=MLPV(m2slab[:, i * NJ:(i + 1) * NJ]),
                            start=(i == 0), stop=(i == NI - 1),
                        )
                    # ---- C: gated message sum, stitched into coord groups ----
                    sg = cp1.tile([NI, NJ], F32, tag="sg")
                    nc.scalar.activation(out=sg, in_=att_ps[0:NI, :],
                                         func=AF.Sigmoid)
                    gmask = cp1.tile([NI, NJ], F32, tag="gmask")
                    nc.vector.tensor_tensor(out=gmask, in0=sg, in1=maskc_sb,
                                            op=OP.mult)
                    msumT = cp1.tile([H, NI], F32, tag="msumT")
                    if BF16MLP:
                        gmask_m = cp1.tile([NI, NJ], BF16, tag="gmb")
                        nc.vector.tensor_copy(gmask_m, gmask)
                    else:
                        gmask_m = gmask

                    def msum_chunk(i4):
                        growc = rowsp.tile([1, 2 * NJ], MLPDT, tag="growc")
                        nc.sync.dma_start(
                            out=growc[:] if BF16MLP else growc[:].bitcast(F32R),
                            in_=gmask_m[2 * i4:2 * (i4 + 1), :] if BF16MLP else
                            gmask_m[2 * i4:2 * (i4 + 1), :].bitcast(F32R))
                        for q in range(2):
                            i = 2 * i4 + q
                            gb = ps_sm.tile([H, NJ], F32, tag="sm")
                            nc.tensor.matmul(
                                gb, lhsT=MLPV(ones_w),
                                rhs=MLPV(growc[0:1, q * NJ:(q + 1) * NJ]),
                                start=True, stop=True)
                            mg = mgp.tile([H, NJ], F32, tag="mg")
                            nc.vector.scalar_tensor_tensor(
                                out=mg, in0=m2slab[:, i * NJ:(i + 1) * NJ],
                                scalar=1.0, in1=gb, op0=OP.mult, op1=OP.mult,
                                accum_out=msumT[:, i:i + 1])

                    # remaining coord groups with msum chunks stitched between
                    ncg = NGRP
                    nch = NI // 2
                    cursor = 0
                    for k, g in enumerate(range(NGRP)):
                        coord_group(g)
                        hi = (k + 1) * nch // ncg
                        while cursor < hi:
                            msum_chunk(cursor)
                            cursor += 1
                    while pend:
                        flush_acc(pend.pop(0), phi_ps[0:NI, :], c3w_w[:, l, :])
                    if DEBUG and l == 0:
                        nc.sync.dma_start(out=dbg["gmask"][:], in_=gmask)
                        nc.sync.dma_start(out=dbg["msum"][:], in_=msumT)
                    nc.sync.dma_start(out=mag_in[l][:], in_=msumT)
                    nc.gpsimd.collective_compute(
                        "AllGather", OP.bypass, replica_groups=rg,
                        ins=[mag_in[l][:]], outs=[mag_out[l][:]],
                    )
                    msumF = cp1.tile([H, NJ], F32, tag="msumF")
                    for r in range(NC):
                        nc.sync.dma_start(
                            out=msumF[:, r * NI:(r + 1) * NI].bitcast(F32R),
                            in_=mag_out[l][r * H:(r + 1) * H, :].bitcast(F32R))
                    # node MLP (all 384 nodes, redundant on every core)
                    z1 = ps_sm.tile([H, NJ], F32, tag="sm")
                    RMM(z1, lhsT=nw1hT_sb[:, l, :], rhs=hT_l,
                        start=True, stop=False)
                    RMM(z1, lhsT=nw1mT_sb[:, l, :], rhs=msumF,
                        start=False, stop=True)
                    z1b = cp1.tile([H, NJ], F32, tag="z1b")
                    nc.vector.tensor_scalar(out=z1b, in0=z1,
                                            scalar1=nb1_sb[:, l:l + 1],
                                            scalar2=None, op0=OP.add)
                    sgn = cp1.tile([H, NJ], F32, tag="sgn")
                    nc.scalar.activation(out=sgn, in_=z1, func=AF.Sigmoid,
                                         bias=nb1_sb[:, l:l + 1], scale=1.0)
                    t1n = cp1.tile([H, NJ], F32, tag="t1n")
                    nc.vector.tensor_tensor(out=t1n[:].bitcast(F32R), in0=z1b,
                                            in1=sgn, op=OP.mult)
                    z2n = ps_sm.tile([H, NJ], F32, tag="sm")
                    RMM(z2n, lhsT=nw2T_sb[:, l, :], rhs=t1n,
                        start=True, stop=True)
                    h_T = cp2.tile([H, NJ], F32, tag="hT")
                    nc.vector.tensor_scalar(out=h_T[:].bitcast(F32R), in0=z2n,
                                            scalar1=nb2_sb[:, l:l + 1],
                                            scalar2=None, op0=OP.add)
                    # local copy of this core's own h rows for the next layer
                    z1m = ps_sm.tile([H, NI], F32, tag="sm")
                    nc.tensor.matmul(z1m, lhsT=nw1hT_sb[:, l, :], rhs=hmy_l,
                                     start=True, stop=False)
                    nc.tensor.matmul(z1m, lhsT=nw1mT_sb[:, l, :], rhs=msumT,
                                     start=False, stop=True)
                    z1bm = cp1.tile([H, NI], F32, tag="z1bm")
                    nc.vector.tensor_scalar(out=z1bm, in0=z1m,
                                            scalar1=nb1_sb[:, l:l + 1],
                                            scalar2=None, op0=OP.add)
                    sgnm = cp1.tile([H, NI], F32, tag="sgnm")
                    nc.scalar.activation(out=sgnm, in_=z1m, func=AF.Sigmoid,
                                         bias=nb1_sb[:, l:l + 1], scale=1.0)
                    t1nm = cp1.tile([H, NI], F32, tag="t1nm")
                    nc.vector.tensor_tensor(out=t1nm[:].bitcast(F32R), in0=z1bm,
                                            in1=sgnm, op=OP.mult)
                    z2m = ps_sm.tile([H, NI], F32, tag="sm")
                    RMM(z2m, lhsT=nw2T_sb[:, l, :], rhs=t1nm,
                        start=True, stop=True)
                    h_my = cp2.tile([H, NI], F32, tag="hmy")
                    nc.vector.tensor_scalar(out=h_my[:].bitcast(F32R), in0=z2m,
                                            scalar1=nb2_sb[:, l:l + 1],
                                            scalar2=None, op0=OP.add)
                else:
                    for g in range(NGRP):
                        coord_group(g)
                    while pend:
                        flush_acc(pend.pop(0), phi_ps[0:NI, :], c3w_w[:, l, :])

                # ---- phi stream + x update ----
                phis = cp1.tile([NI, NJ], F32, tag="phis")
                nc.vector.tensor_scalar(out=phis, in0=phi_ps[0:NI, :],
                                        scalar1=cb3c_sb[:, l:l + 1], scalar2=None,
                                        op0=OP.add)
                s = cp1.tile([NI, NJ], F32, tag="s")
                nc.vector.tensor_tensor(out=s, in0=phis, in1=u, op=OP.mult)
                nc.vector.tensor_tensor(out=s, in0=s, in1=maskc_sb, op=OP.mult)
                xnew = cp2.tile([NI, D], F32, tag="xnew")
                for c in range(D):
                    xm = cp1.tile([NI, NJ], F32, tag="xm")
                    xcol = cp1.tile([NI, 1], F32, tag=f"xcol{c}")
                    nc.vector.scalar_tensor_tensor(
                        out=xm, in0=diff[c], scalar=1.0, in1=s,
                        op0=OP.mult, op1=OP.mult, accum_out=xcol)
                    nc.vector.tensor_tensor(out=xnew[:, c:c + 1], in0=xcol,
                                            in1=x_my[:, c:c + 1], op=OP.add)
                if DEBUG and l == 0:
                    nc.sync.dma_start(out=dbg["phis"][:], in_=phis)
                    nc.sync.dma_start(out=dbg["x1"][:], in_=xnew)
                    nc.sync.dma_start(out=dbg["h1"][:], in_=h_T)
                if not last:
                    nc.sync.dma_start(out=xag_in[l].rearrange("c n -> n c"),
                                      in_=xnew)
                else:
                    nc.sync.dma_start(out=xag_in[l][:], in_=xnew)
                nc.gpsimd.collective_compute(
                    "AllGather", OP.bypass, replica_groups=rg,
                    ins=[xag_in[l][:]], outs=[xag_out[l][:]],
                )
                if not last:
                    x_my = xnew
                else:
                    nc.sync.dma_start(out=o_x[:], in_=xag_out[l][:])

    nc.finalize()
    return nc


def _prep_inputs(inputs):
    """Host-side prep: per-core input maps from full arrays."""
    f = lambda a: np.ascontiguousarray(np.asarray(a), dtype=np.float32)
    x_inp = f(inputs["x_inp"])
    emb_w = f(inputs["emb_w"])
    emb_b = f(inputs["emb_b"])
    coord_w1 = f(inputs["coord_w1"])
    coord_b1 = f(inputs["coord_b1"])
    coord_w2 = f(inputs["coord_w2"])
    coord_b2 = f(inputs["coord_b2"])
    coord_w3 = f(inputs["coord_w3"])
    coord_b3 = f(inputs["coord_b3"])
    edge_w1 = f(inputs["edge_w1"])
    edge_b1 = f(inputs["edge_b1"])
    edge_w2 = f(inputs["edge_w2"])
    edge_b2 = f(inputs["edge_b2"])
    node_w1 = f(inputs["node_w1"])
    node_b1 = f(inputs["node_b1"])
    node_w2 = f(inputs["node_w2"])
    node_b2 = f(inputs["node_b2"])
    att_w = f(inputs["att_w"])

    x0 = x_inp.reshape(N, D)
    eye = np.eye(N, dtype=np.float32)

    def stackT(w, lo, hi):
        return np.ascontiguousarray(
            np.stack([w[l, :, lo:hi].T for l in range(w.shape[0])]))

    def win(w3):
        nl = w3.shape[0]
        out = np.zeros((nl, H, 2 * NI - 1), np.float32)
        out[:, :, NI - 1] = w3[:, 0, :]
        return out

    shared = dict(
        x0rows=np.ascontiguousarray(x0.T.reshape(1, D * N)),
        c1hiT=stackT(coord_w1, 0, H),
        c1hjT=stackT(coord_w1, H, 2 * H),
        c1drep=np.ascontiguousarray(np.tile(coord_w1[:, :, 2 * H], (1, 16))),
        cb1=np.ascontiguousarray(coord_b1.T),
        c2T=np.ascontiguousarray(np.stack([coord_w2[l].T for l in range(L)])),
        cb2=np.ascontiguousarray(coord_b2.T),
        c3w=win(coord_w3),
        cb3c=np.ascontiguousarray(
            np.broadcast_to(coord_b3[:, 0][None, :], (NI, L))),
        e1hiT=stackT(edge_w1, 0, H),
        e1hjT=stackT(edge_w1, H, 2 * H),
        e1drep=np.ascontiguousarray(np.tile(edge_w1[:, :, 2 * H], (1, 16))),
        ones6k=np.ones((1, 16 * NJ), np.float32),
        eb1=np.ascontiguousarray(edge_b1.T),
        e2T=np.ascontiguousarray(np.stack([edge_w2[l].T for l in range(L - 1)])),
        eb2=np.ascontiguousarray(edge_b2.T),
        attw=win(att_w),
        nw1hT=stackT(node_w1, 0, H),
        nw1mT=stackT(node_w1, H, 2 * H),
        nb1=np.ascontiguousarray(node_b1.T),
        nw2T=np.ascontiguousarray(np.stack([node_w2[l].T for l in range(L - 1)])),
        nb2=np.ascontiguousarray(node_b2.T),
        ones128=np.ones((1, H), np.float32),
    )
    in_maps = []
    for c in range(NC):
        m = dict(shared)
        m["embw"] = np.ascontiguousarray(
            emb_w[c * EMB_ROWS:(c + 1) * EMB_ROWS, :])
        m["embbT"] = np.ascontiguousarray(
            emb_b[c * EMB_ROWS:(c + 1) * EMB_ROWS].reshape(NI, H).T)
        m["x0my"] = np.ascontiguousarray(x0[c * NI:(c + 1) * NI, :])
        m["maskc"] = np.ascontiguousarray(1.0 - eye[c * NI:(c + 1) * NI, :])
        m["eyec"] = np.ascontiguousarray(eye[c * NI:(c + 1) * NI, :])
        in_maps.append(m)
    return in_maps


def _run(inputs, trace=False, **kw):
    from concourse.bass_utils import run_bass_kernel_spmd
    if "nc" not in _cache:
        _cache["nc"] = _build_nc()
    in_maps = _prep_inputs(inputs)
    return run_bass_kernel_spmd(_cache["nc"], in_maps, list(range(NC)),
                                trace=trace, **kw)


def kernel(**inputs) -> np.ndarray:
    res = _run(inputs)
    return np.asarray(res.results[0]["o_x"], dtype=np.float32).reshape(N * D)



# revision 13
# speedup vs baseline: 1.2902x; 1.2902x over previous
"""EGNN (N=384, D=3, H=128, L=4) Bass kernel for 8 TRN2 NeuronCores.

Sharding: rows of the N x N edge grid split across 8 cores (48 rows each).
Per layer each core computes its row-block of the coord/edge MLPs and
row-sums, AllGathers the per-node x updates and msum rows; the node MLP is
computed redundantly per core. The embedding row-sum of emb_w (the dominant
HBM traffic) is sharded 1/8 per core.

Key perf structure (PE observed pinned at 1.2 GHz = 1 cyc/row for both
f32r and bf16, so coord-pass f32r precision is free):
- G=3 row groups; group g covers rows {g, 16+g, 32+g} so the three rank-2
  [w1d; A_i] matmuls land in PE row-quadrants 0/32/64 and run concurrently.
- Edge pass bf16 (halves SBUF for the m2 slab), coord pass f32r.
- Gate sigmoid via tanh (same ACT table as silu -> zero table swaps);
  sqrt via bit-trick + Newton using reciprocal_approx_fast on the DVE.
- Gated msum: gate rows partition-broadcast via DRAM stride-0 DMA, then a
  fused scalar_tensor_tensor accumulation per row (no PE, no PSUM).
- Embedding: 8 large DMAs issued up-front on 3 queues; layer-0 coordinate
  prep hoisted before the reduces to fill the DVE while DMAs stream.
"""
import os
import numpy as np

KTRUNC = int(os.environ.get("KTRUNC", "0"))

N, D, H, L = 384, 3, 128, 4
NC = 8
NI = N // NC          # 48 rows per core
NJ = N                # 384 cols
G = 3                 # rows per group (one per PE row-quadrant)
NGRP = NI // G        # 16 groups
EB = 6                # nodes per embedding DMA
NEMB = NI // EB       # 8 embedding tiles
EMB_ROWS = N * H // NC

QUAKE = 0x1FBD1DF5

_cache = {}


def _build_nc():
    import concourse.bass as bass
    import concourse.bacc as bacc
    import concourse.tile as tile
    from concourse import mybir

    F32 = mybir.dt.float32
    F32R = mybir.dt.float32r
    BF16 = mybir.dt.bfloat16
    I32 = mybir.dt.int32
    AF = mybir.ActivationFunctionType
    OP = mybir.AluOpType

    nc = bacc.Bacc(None, target_bir_lowering=False)

    def RMM(out, lhsT, rhs, **kw):
        nc.tensor.matmul(out, lhsT=lhsT.bitcast(F32R), rhs=rhs.bitcast(F32R), **kw)

    def P(name, shape, dt=F32):
        return nc.declare_dram_parameter(name, list(shape), dt, isOutput=False)

    # per-core inputs
    embw = P("embw", (EMB_ROWS, NJ))
    embbT = P("embbT", (H, NI))
    x0my = P("x0my", (NI, D))
    maskc = P("maskc", (NI, NJ))
    eyec = P("eyec", (NI, NJ))
    # shared inputs (f32)
    x0rows = P("x0rows", (1, D * NJ))
    c1hiT = P("c1hiT", (L, H, H))
    c1hjT = P("c1hjT", (L, H, H))
    cb1 = P("cb1", (H, L))
    cb2 = P("cb2", (H, L))
    c2T = P("c2T", (L, H, H))
    c3w = P("c3w", (L, H, 2 * NI - 1))
    cb3c = P("cb3c", (NI, L))
    e1hiT = P("e1hiT", (L - 1, H, H))
    eb1 = P("eb1", (H, L - 1))
    eb2 = P("eb2", (H, L - 1))
    nw1hT = P("nw1hT", (L - 1, H, H))
    nw1mT = P("nw1mT", (L - 1, H, H))
    nb1 = P("nb1", (H, L - 1))
    nw2T = P("nw2T", (L - 1, H, H))
    nb2 = P("nb2", (H, L - 1))
    cdrep = P("cdrep", (L, 16 * H))
    edrep = P("edrep", (L - 1, 16 * H))
    ones6k = P("ones6k", (1, 16 * NJ))
    # shared inputs (bf16, edge pass)
    e1hjT_b = P("e1hjT_b", (L - 1, H, H), BF16)
    e2T_b = P("e2T_b", (L - 1, H, H), BF16)
    attw_b = P("attw_b", (L - 1, H, 2 * NI - 1), BF16)

    o_x = nc.declare_dram_parameter("o_x", [N, D], F32, isOutput=True)

    # DRAM internals
    gdram = [nc.dram_tensor(f"gdram{l}", [NI * NJ], BF16) for l in range(L - 1)]
    hag_in = nc.dram_tensor("hag_in", [H, NI], F32)
    hag_out = nc.dram_tensor("hag_out", [NC * H, NI], F32, addr_space="Shared")
    xag_in = [nc.dram_tensor(f"xag_in{l}", [D, NI], F32) for l in range(L - 1)]
    xag_in.append(nc.dram_tensor("xag_in3", [NI, D], F32))
    xag_out = [nc.dram_tensor(f"xag_out{l}", [NC, D, NI], F32, addr_space="Shared")
               for l in range(L - 1)]
    xag_out.append(nc.dram_tensor("xag_out3", [N, D], F32, addr_space="Shared"))
    mag_in = [nc.dram_tensor(f"mag_in{l}", [H, NI], F32) for l in range(L - 1)]
    mag_out = [nc.dram_tensor(f"mag_out{l}", [NC * H, NI], F32, addr_space="Shared")
               for l in range(L - 1)]
    rg = [list(range(NC))]

    with tile.TileContext(nc) as tc:
        with (
            tc.tile_pool(name="consts", bufs=1) as consts,
            tc.tile_pool(name="embp", bufs=3) as embp,
            tc.tile_pool(name="cp1", bufs=1) as cp1,
            tc.tile_pool(name="cp2", bufs=2) as cp2,
            tc.tile_pool(name="work", bufs=2) as work,
            tc.tile_pool(name="slab", bufs=1) as slabp,
            tc.tile_pool(name="gbp", bufs=4) as gbp,
            tc.tile_pool(name="ps_mlp", bufs=2, space="PSUM") as ps_mlp,
            tc.tile_pool(name="ps_acc", bufs=1, space="PSUM") as ps_acc,
            tc.tile_pool(name="ps_nd", bufs=1, space="PSUM") as ps_nd,
        ):
            # ---- phase 0: embedding DMAs first, across 3 queues ----
            emb_tiles = []
            qrot = [nc.sync, nc.scalar, nc.gpsimd]
            for t in range(NEMB):
                et = embp.tile([H, EB, NJ], F32, tag="embt")
                qrot[t % 3].dma_start(
                    out=et,
                    in_=embw[t * EB * H:(t + 1) * EB * H, :]
                    .rearrange("(a p) j -> p a j", p=H))
                emb_tiles.append(et)

            # ---- constants (scalar queue; ACT is idle during emb) ----
            def load(pname, ap_in, shape, dt=F32, rnd=False):
                t = consts.tile(list(shape), dt, tag=pname)
                o = t[:].bitcast(F32R) if rnd else t
                nc.scalar.dma_start(
                    out=o, in_=ap_in.bitcast(F32R) if rnd else ap_in)
                return t

            x0my_sb = load("x0my", x0my[:], (NI, D))
            embbT_sb = load("embbT", embbT[:], (H, NI))
            maskc_sb = load("maskc", maskc[:], (NI, NJ))
            eyec_sb = load("eyec", eyec[:], (NI, NJ))
            c1hiT_sb = load("c1hiT", c1hiT.rearrange("l p x -> p l x"), (H, L, H),
                            rnd=True)
            c1hjT_sb = load("c1hjT", c1hjT.rearrange("l p x -> p l x"), (H, L, H),
                            rnd=True)
            c2T_sb = load("c2T", c2T.rearrange("l p x -> p l x"), (H, L, H),
                          rnd=True)
            c3w_sb = load("c3w", c3w.rearrange("l p x -> p l x"),
                          (H, L, 2 * NI - 1), rnd=True)
            cb1_sb = load("cb1", cb1[:], (H, L))
            cb2_sb = load("cb2", cb2[:], (H, L))
            cb3c_sb = load("cb3c", cb3c[:], (NI, L))
            e1hiT_sb = load("e1hiT", e1hiT.rearrange("l p x -> p l x"),
                            (H, L - 1, H), rnd=True)
            eb1_sb = load("eb1", eb1[:], (H, L - 1))
            eb2_sb = load("eb2", eb2[:], (H, L - 1))
            nw1hT_sb = load("nw1hT", nw1hT.rearrange("l p x -> p l x"),
                            (H, L - 1, H), rnd=True)
            nw1mT_sb = load("nw1mT", nw1mT.rearrange("l p x -> p l x"),
                            (H, L - 1, H), rnd=True)
            nb1_sb = load("nb1", nb1[:], (H, L - 1))
            nw2T_sb = load("nw2T", nw2T.rearrange("l p x -> p l x"),
                           (H, L - 1, H), rnd=True)
            nb2_sb = load("nb2", nb2[:], (H, L - 1))
            cdrep_sb = load("cdrep", cdrep[:], (L, 16 * H), rnd=True)
            edrep_sb = load("edrep", edrep[:], (L - 1, 16 * H), rnd=True)
            e1hjT_w = load("e1hjT_b", e1hjT_b.rearrange("l p x -> p l x"),
                           (H, L - 1, H), BF16)
            e2T_w = load("e2T_b", e2T_b.rearrange("l p x -> p l x"),
                         (H, L - 1, H), BF16)
            attw_w = load("attw_b", attw_b.rearrange("l p x -> p l x"),
                          (H, L - 1, 2 * NI - 1), BF16)

            # d2/ones interleaved stripes (f32r): rows 32k = d2 rows
            # 16k..16k+15 flattened, rows 32k+1 = ones.
            d2ones = consts.tile([66, 16 * NJ], F32, tag="d2ones")
            combC = consts.tile([66, 16 * H], F32, tag="combC")
            combE = consts.tile([66, 16 * H], F32, tag="combE")
            for k in range(3):
                nc.scalar.dma_start(
                    out=d2ones[32 * k + 1:32 * k + 2, :].bitcast(F32R),
                    in_=ones6k[:].bitcast(F32R))

            def coord_prep(l, x_my):
                """xb loads + diff/d2/u chain + d2 stripes for layer l."""
                diff = []
                for c in range(D):
                    xb = cp1.tile([NI, NJ], F32, tag=f"xb{c}")
                    if l == 0:
                        bsrc = bass.AP(tensor=x0rows, offset=c * NJ,
                                       ap=[[0, NI], [1, NJ]])
                    else:
                        bsrc = bass.AP(tensor=xag_out[l - 1], offset=c * NI,
                                       ap=[[0, NI], [D * NI, NC], [1, NI]])
                    nc.sync.dma_start(out=xb, in_=bsrc)
                    dc = cp2.tile([NI, NJ], F32, tag=f"diff{c}")
                    nc.vector.tensor_scalar(
                        out=dc, in0=xb, scalar1=x_my[:, c:c + 1], scalar2=None,
                        op0=OP.subtract)
                    diff.append(dc)
                d2 = cp1.tile([NI, NJ], F32, tag="d2")
                tmp = cp1.tile([NI, NJ], F32, tag="ctmp")
                nc.vector.tensor_tensor(out=d2, in0=diff[0], in1=diff[0],
                                        op=OP.mult)
                nc.vector.tensor_tensor(out=tmp, in0=diff[1], in1=diff[1],
                                        op=OP.mult)
                nc.vector.tensor_tensor(out=d2, in0=d2, in1=tmp, op=OP.add)
                nc.vector.tensor_tensor(out=tmp, in0=diff[2], in1=diff[2],
                                        op=OP.mult)
                nc.vector.tensor_tensor(out=d2, in0=d2, in1=tmp, op=OP.add)
                for k in range(3):
                    nc.sync.dma_start(
                        out=d2ones[32 * k:32 * k + 1, :].bitcast(F32R),
                        in_=d2[16 * k:16 * (k + 1), :].bitcast(F32R))
                d2s = cp1.tile([NI, NJ], F32, tag="d2s")
                nc.vector.tensor_tensor(out=d2s, in0=d2, in1=eyec_sb, op=OP.add)
                # sqrt via bit-trick seed + 1 Newton step (approx recip)
                sq = cp1.tile([NI, NJ], F32, tag="sq")
                with nc.allow_low_precision(reason="bit-trick sqrt"):
                    nc.vector.tensor_scalar(
                        out=sq.bitcast(I32), in0=d2s.bitcast(I32),
                        scalar1=1, scalar2=None, op0=OP.logical_shift_right)
                    nc.vector.tensor_scalar(
                        out=sq.bitcast(I32), in0=sq.bitcast(I32),
                        scalar1=QUAKE, scalar2=None, op0=OP.add)
                nc.vector.reciprocal_approx_fast(out=tmp, in_=sq)
                nc.vector.tensor_tensor(out=tmp, in0=d2s, in1=tmp, op=OP.mult)
                nc.vector.tensor_tensor(out=sq, in0=sq, in1=tmp, op=OP.add)
                nc.vector.tensor_scalar(out=sq, in0=sq, scalar1=0.5,
                                        scalar2=None, op0=OP.mult)
                nc.vector.tensor_scalar(out=sq, in0=sq, scalar1=1.0,
                                        scalar2=None, op0=OP.add)
                u = cp2.tile([NI, NJ], F32, tag="u")
                nc.vector.reciprocal_approx_fast(out=u, in_=sq)
                nc.vector.tensor_tensor(out=u, in0=u, in1=maskc_sb, op=OP.mult)
                return diff, u

            prep0 = coord_prep(0, x0my_sb)

            # ---- embedding reduce + h AllGather ----
            hT0 = cp1.tile([H, NI], F32, tag="hT0")
            for t in range(NEMB):
                with nc.allow_low_precision(reason="f32r storage, f32 accum"):
                    nc.vector.tensor_reduce(
                        out=hT0[:, t * EB:(t + 1) * EB].bitcast(F32R),
                        in_=emb_tiles[t],
                        axis=mybir.AxisListType.X, op=OP.add)
            nc.vector.tensor_tensor(out=hT0[:].bitcast(F32R), in0=hT0,
                                    in1=embbT_sb, op=OP.add)
            nc.gpsimd.dma_start(out=hag_in[:], in_=hT0)
            nc.gpsimd.collective_compute(
                "AllGather", OP.bypass, replica_groups=rg,
                ins=[hag_in[:]], outs=[hag_out[:]])
            h_T = cp2.tile([H, NJ], F32, tag="hT")
            for r in range(NC):
                nc.sync.dma_start(
                    out=h_T[:, r * NI:(r + 1) * NI].bitcast(F32R),
                    in_=hag_out[r * H:(r + 1) * H, :].bitcast(F32R))
            h_my = hT0
            hT_bf = cp2.tile([H, NJ], BF16, tag="hTb")
            nc.vector.tensor_copy(hT_bf, h_T)

            x_my = x0my_sb

            for l in range(L):
                if KTRUNC == 1 or (KTRUNC and l > 0):
                    break
                last = l == L - 1
                do_gate = (not last) and KTRUNC in (0, 3, 4)
                do_coord = KTRUNC in (0, 3, 4)
                do_node = (not last) and KTRUNC in (0, 4)

                diff, u = prep0 if l == 0 else coord_prep(l, x_my)

                # A-terms: At[i, :] = (W1hi @ h_i), striped into comb pairs
                def make_comb(hiT, wdrep_row, comb, atag):
                    At_ps = ps_nd.tile([NI, H], F32, tag="nd")
                    RMM(At_ps, lhsT=h_my, rhs=hiT, start=True, stop=True)
                    At = cp1.tile([NI, H], F32, tag=atag)
                    nc.vector.tensor_copy(At[:].bitcast(F32R), At_ps)
                    for k in range(3):
                        nc.sync.dma_start(
                            out=comb[32 * k:32 * k + 1, :].bitcast(F32R),
                            in_=wdrep_row.bitcast(F32R))
                        nc.sync.dma_start(
                            out=comb[32 * k + 1:32 * k + 2, :].bitcast(F32R),
                            in_=At[16 * k:16 * (k + 1), :].bitcast(F32R))

                make_comb(c1hiT_sb[:, l, :], cdrep_sb[l:l + 1, :], combC, "Atc")
                if not last:
                    make_comb(e1hiT_sb[:, l, :], edrep_sb[l:l + 1, :], combE,
                              "Ate")

                # group g covers rows {g, 16+g, 32+g}: the three rank-2
                # matmuls hit PE row-quadrants 0/32/64 and run concurrently
                def mm_d(pre, comb, g):
                    for r in range(G):
                        nc.tensor.matmul(
                            pre[:, r * 512:r * 512 + NJ],
                            lhsT=comb[32 * r:32 * r + 2,
                                      g * H:(g + 1) * H].bitcast(F32R),
                            rhs=d2ones[32 * r:32 * r + 2,
                                       g * NJ:(g + 1) * NJ].bitcast(F32R),
                            start=False, stop=True)

                # ---- edge pass (bf16) ----
                if not last:
                    att_ps = ps_acc.tile([H, NJ], F32, tag="acc")
                    m2slab = slabp.tile([H, NI * NJ], BF16, tag="m2")
                    for g in range(NGRP):
                        pre = ps_mlp.tile([H, G * 512], F32, tag="mlp")
                        for r in range(G):
                            nc.tensor.matmul(
                                pre[:, r * 512:r * 512 + NJ],
                                lhsT=e1hjT_w[:, l, :], rhs=hT_bf,
                                start=True, stop=False)
                        mm_d(pre, combE, g)
                        t1 = work.tile([H, G * NJ], BF16, tag="t1")
                        nc.scalar.activation(
                            out=t1.rearrange("p (r c) -> p r c", r=G),
                            in_=pre.rearrange("p (r c) -> p r c", r=G)[:, :, 0:NJ],
                            func=AF.Silu, bias=eb1_sb[:, l:l + 1], scale=1.0)
                        z2 = ps_mlp.tile([H, G * 512], F32, tag="mlp")
                        nc.tensor.matmul(z2[:, 0:512], lhsT=e2T_w[:, l, :],
                                         rhs=t1[:, 0:512], start=True, stop=True)
                        nc.tensor.matmul(z2[:, 512:1024], lhsT=e2T_w[:, l, :],
                                         rhs=t1[:, 512:1024], start=True,
                                         stop=True)
                        nc.tensor.matmul(z2[:, 1024:1152], lhsT=e2T_w[:, l, :],
                                         rhs=t1[:, 1024:1152], start=True,
                                         stop=True)
                        nc.scalar.activation(
                            out=m2slab[:, g * G * NJ:(g + 1) * G * NJ],
                            in_=z2[:, 0:G * NJ],
                            func=AF.Silu, bias=eb2_sb[:, l:l + 1], scale=1.0)
                        for r in range(G):
                            i = 16 * r + g
                            s = G * g + r
                            nc.tensor.matmul(
                                att_ps[0:NI, :],
                                lhsT=attw_w[:, l, (NI - 1) - i:(2 * NI - 1) - i],
                                rhs=m2slab[:, s * NJ:(s + 1) * NJ],
                                start=(s == 0), stop=(s == NI - 1))

                    # gate: exact sigmoid via tanh (same ACT table as silu)
                    if not do_gate:
                        continue
                    sg = cp1.tile([NI, NJ], F32, tag="sg")
                    nc.scalar.activation(out=sg, in_=att_ps[0:NI, :],
                                         func=AF.Tanh, bias=0.0, scale=0.5)
                    gmask = cp1.tile([NI, NJ], F32, tag="gmask")
                    nc.vector.tensor_scalar(out=gmask, in0=sg, scalar1=0.5,
                                            scalar2=0.5, op0=OP.mult,
                                            op1=OP.add)
                    nc.vector.tensor_tensor(out=gmask, in0=gmask, in1=maskc_sb,
                                            op=OP.mult)
                    gmb = cp1.tile([NI, NJ], BF16, tag="gmb")
                    nc.vector.tensor_copy(gmb, gmask)
                    nc.sync.dma_start(out=gdram[l][:], in_=gmb)
                    # partition-broadcast gate rows via DRAM stride-0 reads,
                    # in slab-slot order s = G*g + r  ->  grid row 16*r + g
                    gb_tiles = [None] * NI
                    for g in range(NGRP):
                        for r in range(G):
                            i = 16 * r + g
                            s = G * g + r
                            gb = gbp.tile([H, NJ], BF16, tag="gb")
                            eng = nc.sync if s % 2 == 0 else nc.gpsimd
                            eng.dma_start(
                                out=gb,
                                in_=bass.AP(tensor=gdram[l], offset=i * NJ,
                                            ap=[[0, H], [1, NJ]]))
                            gb_tiles[s] = gb
                    msumT = cp1.tile([H, NI], F32, tag="msumT")
                    mgs = cp1.tile([H, NJ], BF16, tag="mgs")

                # ---- coord pass (f32r) ----
                if do_coord:
                    phi_ps = ps_acc.tile([H, NJ], F32, tag="acc")

                    def msum_row(s):
                        i = 16 * (s % G) + s // G
                        nc.vector.scalar_tensor_tensor(
                            out=mgs, in0=m2slab[:, s * NJ:(s + 1) * NJ],
                            scalar=1.0, in1=gb_tiles[s],
                            op0=OP.mult, op1=OP.mult,
                            accum_out=msumT[:, i:i + 1])

                    for g in range(NGRP):
                        pre = ps_mlp.tile([H, G * 512], F32, tag="mlp")
                        for r in range(G):
                            RMM(pre[:, r * 512:r * 512 + NJ],
                                lhsT=c1hjT_sb[:, l, :], rhs=h_T,
                                start=True, stop=False)
                        mm_d(pre, combC, g)
                        t1c = work.tile([H, G * NJ], F32, tag="t1c")
                        nc.scalar.activation(
                            out=t1c[:].bitcast(F32R)
                            .rearrange("p (r c) -> p r c", r=G),
                            in_=pre.rearrange("p (r c) -> p r c", r=G)[:, :, 0:NJ],
                            func=AF.Silu, bias=cb1_sb[:, l:l + 1], scale=1.0)
                        z2 = ps_mlp.tile([H, G * 512], F32, tag="mlp")
                        RMM(z2[:, 0:512], lhsT=c2T_sb[:, l, :],
                            rhs=t1c[:, 0:512], start=True, stop=True)
                        RMM(z2[:, 512:1024], lhsT=c2T_sb[:, l, :],
                            rhs=t1c[:, 512:1024], start=True, stop=True)
                        RMM(z2[:, 1024:1152], lhsT=c2T_sb[:, l, :],
                            rhs=t1c[:, 1024:1152], start=True, stop=True)
                        t2c = work.tile([H, G * NJ], F32, tag="t2c")
                        nc.scalar.activation(
                            out=t2c[:].bitcast(F32R),
                            in_=z2[:, 0:G * NJ],
                            func=AF.Silu, bias=cb2_sb[:, l:l + 1], scale=1.0)
                        for r in range(G):
                            i = 16 * r + g
                            s = G * g + r
                            RMM(phi_ps[0:NI, :],
                                lhsT=c3w_sb[:, l,
                                            (NI - 1) - i:(2 * NI - 1) - i],
                                rhs=t2c[:, r * NJ:(r + 1) * NJ],
                                start=(s == 0), stop=(s == NI - 1))
                        if do_gate:
                            for r in range(G):
                                msum_row(G * g + r)

                if do_node:
                    # msum AllGather + node MLP (overlaps late coord groups)
                    nc.gpsimd.dma_start(out=mag_in[l][:], in_=msumT)
                    nc.gpsimd.collective_compute(
                        "AllGather", OP.bypass, replica_groups=rg,
                        ins=[mag_in[l][:]], outs=[mag_out[l][:]])
                    msumF = cp1.tile([H, NJ], F32, tag="msumF")
                    for r in range(NC):
                        nc.gpsimd.dma_start(
                            out=msumF[:, r * NI:(r + 1) * NI].bitcast(F32R),
                            in_=mag_out[l][r * H:(r + 1) * H, :].bitcast(F32R))
                    z1 = ps_nd.tile([H, 512], F32, tag="nd")
                    RMM(z1[:, 0:NJ], lhsT=nw1hT_sb[:, l, :], rhs=h_T,
                        start=True, stop=False)
                    RMM(z1[:, 0:NJ], lhsT=nw1mT_sb[:, l, :], rhs=msumF,
                        start=False, stop=True)
                    t1n = cp1.tile([H, NJ], F32, tag="t1n")
                    nc.scalar.activation(out=t1n[:].bitcast(F32R),
                                         in_=z1[:, 0:NJ], func=AF.Silu,
                                         bias=nb1_sb[:, l:l + 1], scale=1.0)
                    z2n = ps_nd.tile([H, 512], F32, tag="nd")
                    RMM(z2n[:, 0:NJ], lhsT=nw2T_sb[:, l, :], rhs=t1n,
                        start=True, stop=True)
                    h_T = cp2.tile([H, NJ], F32, tag="hT")
                    nc.vector.tensor_scalar(out=h_T[:].bitcast(F32R),
                                            in0=z2n[:, 0:NJ],
                                            scalar1=nb2_sb[:, l:l + 1],
                                            scalar2=None, op0=OP.add)
                    if l < L - 2:
                        hT_bf = cp2.tile([H, NJ], BF16, tag="hTb")
                        nc.vector.tensor_copy(hT_bf, h_T)
                    # local copy of this core's own h rows
                    z1m = ps_nd.tile([H, 512], F32, tag="nd")
                    nc.tensor.matmul(z1m[:, 0:NI], lhsT=nw1hT_sb[:, l, :],
                                     rhs=h_my, start=True, stop=False)
                    nc.tensor.matmul(z1m[:, 0:NI], lhsT=nw1mT_sb[:, l, :],
                                     rhs=msumT, start=False, stop=True)
                    t1m = cp1.tile([H, NI], F32, tag="t1m")
                    nc.scalar.activation(out=t1m, in_=z1m[:, 0:NI],
                                         func=AF.Silu,
                                         bias=nb1_sb[:, l:l + 1], scale=1.0)
                    z2m = ps_nd.tile([H, 512], F32, tag="nd")
                    nc.tensor.matmul(z2m[:, 0:NI], lhsT=nw2T_sb[:, l, :],
                                     rhs=t1m, start=True, stop=True)
                    h_my = cp2.tile([H, NI], F32, tag="hmy")
                    nc.vector.tensor_scalar(out=h_my[:].bitcast(F32R),
                                            in0=z2m[:, 0:NI],
                                            scalar1=nb2_sb[:, l:l + 1],
                                            scalar2=None, op0=OP.add)

                if do_coord:
                    # ---- phi stream + x update ----
                    phis = cp1.tile([NI, NJ], F32, tag="phis")
                    nc.vector.tensor_scalar(out=phis, in0=phi_ps[0:NI, :],
                                            scalar1=cb3c_sb[:, l:l + 1],
                                            scalar2=None, op0=OP.add)
                    s = cp1.tile([NI, NJ], F32, tag="s")
                    nc.vector.tensor_tensor(out=s, in0=phis, in1=u, op=OP.mult)
                    xnew = cp2.tile([NI, D], F32, tag="xnew")
                    xms = cp1.tile([NI, NJ], F32, tag="xms")
                    for c in range(D):
                        xcol = cp1.tile([NI, 1], F32, tag=f"xcol{c}")
                        nc.vector.scalar_tensor_tensor(
                            out=xms, in0=diff[c], scalar=1.0, in1=s,
                            op0=OP.mult, op1=OP.mult, accum_out=xcol)
                        nc.vector.tensor_tensor(
                            out=xnew[:, c:c + 1], in0=xcol,
                            in1=x_my[:, c:c + 1], op=OP.add)
                    if not last:
                        nc.gpsimd.dma_start(
                            out=xag_in[l].rearrange("c n -> n c"), in_=xnew)
                    else:
                        nc.gpsimd.dma_start(out=xag_in[l][:], in_=xnew)
                    nc.gpsimd.collective_compute(
                        "AllGather", OP.bypass, replica_groups=rg,
                        ins=[xag_in[l][:]], outs=[xag_out[l][:]])
                    if not last:
                        x_my = xnew
                    else:
                        nc.sync.dma_start(out=o_x[:], in_=xag_out[l][:])

            if KTRUNC:
                nc.sync.dma_start(out=o_x[0:NI, :], in_=x0my_sb)

    nc.finalize()
    return nc


def _prep_inputs(inputs):
    import ml_dtypes
    BF = ml_dtypes.bfloat16
    f = lambda a: np.ascontiguousarray(np.asarray(a), dtype=np.float32)
    b = lambda a: np.ascontiguousarray(np.asarray(a, dtype=np.float32)
                                       .astype(BF))
    x_inp = f(inputs["x_inp"])
    emb_w = f(inputs["emb_w"])
    emb_b = f(inputs["emb_b"])
    coord_w1 = f(inputs["coord_w1"])
    coord_b1 = f(inputs["coord_b1"])
    coord_w2 = f(inputs["coord_w2"])
    coord_b2 = f(inputs["coord_b2"])
    coord_w3 = f(inputs["coord_w3"])
    coord_b3 = f(inputs["coord_b3"])
    edge_w1 = f(inputs["edge_w1"])
    edge_b1 = f(inputs["edge_b1"])
    edge_w2 = f(inputs["edge_w2"])
    edge_b2 = f(inputs["edge_b2"])
    node_w1 = f(inputs["node_w1"])
    node_b1 = f(inputs["node_b1"])
    node_w2 = f(inputs["node_w2"])
    node_b2 = f(inputs["node_b2"])
    att_w = f(inputs["att_w"])

    x0 = x_inp.reshape(N, D)
    eye = np.eye(N, dtype=np.float32)

    def stackT(w, lo, hi):
        return np.ascontiguousarray(
            np.stack([w[l, :, lo:hi].T for l in range(w.shape[0])]))

    def win(w3):
        nl = w3.shape[0]
        out = np.zeros((nl, H, 2 * NI - 1), np.float32)
        out[:, :, NI - 1] = w3[:, 0, :]
        return out

    shared = dict(
        x0rows=np.ascontiguousarray(x0.T.reshape(1, D * N)),
        c1hiT=stackT(coord_w1, 0, H),
        c1hjT=stackT(coord_w1, H, 2 * H),
        cb1=np.ascontiguousarray(coord_b1.T),
        cb2=np.ascontiguousarray(coord_b2.T),
        c2T=np.ascontiguousarray(np.stack([coord_w2[l].T for l in range(L)])),
        c3w=win(coord_w3),
        cb3c=np.ascontiguousarray(
            np.broadcast_to(coord_b3[:, 0][None, :], (NI, L))),
        e1hiT=stackT(edge_w1, 0, H),
        eb1=np.ascontiguousarray(edge_b1.T),
        eb2=np.ascontiguousarray(edge_b2.T),
        nw1hT=stackT(node_w1, 0, H),
        nw1mT=stackT(node_w1, H, 2 * H),
        nb1=np.ascontiguousarray(node_b1.T),
        nw2T=np.ascontiguousarray(np.stack([node_w2[l].T for l in range(L - 1)])),
        nb2=np.ascontiguousarray(node_b2.T),
        cdrep=np.ascontiguousarray(np.tile(coord_w1[:, :, 2 * H], (1, 16))),
        edrep=np.ascontiguousarray(np.tile(edge_w1[:, :, 2 * H], (1, 16))),
        ones6k=np.ones((1, 16 * NJ), np.float32),
        e1hjT_b=b(stackT(edge_w1, H, 2 * H)),
        e2T_b=b(np.stack([edge_w2[l].T for l in range(L - 1)])),
        attw_b=b(win(att_w)),
    )
    in_maps = []
    for c in range(NC):
        m = dict(shared)
        m["embw"] = np.ascontiguousarray(
            emb_w[c * EMB_ROWS:(c + 1) * EMB_ROWS, :])
        m["embbT"] = np.ascontiguousarray(
            emb_b[c * EMB_ROWS:(c + 1) * EMB_ROWS].reshape(NI, H).T)
        m["x0my"] = np.ascontiguousarray(x0[c * NI:(c + 1) * NI, :])
        m["maskc"] = np.ascontiguousarray(1.0 - eye[c * NI:(c + 1) * NI, :])
        m["eyec"] = np.ascontiguousarray(eye[c * NI:(c + 1) * NI, :])
        in_maps.append(m)
    return in_maps


def _run(inputs, trace=False, **kw):
    from concourse.bass_utils import run_bass_kernel_spmd
    if "nc" not in _cache:
        _cache["nc"] = _build_nc()
    in_maps = _prep_inputs(inputs)
    return run_bass_kernel_spmd(_cache["nc"], in_maps, list(range(NC)),
                                trace=trace, **kw)


def kernel(**inputs) -> np.ndarray:
    res = _run(inputs)
    return np.asarray(res.results[0]["o_x"], dtype=np.float32).reshape(N * D)


# revision 14
# speedup vs baseline: 1.4542x; 1.1272x over previous
"""EGNN (N=384, D=3, H=128, L=4) Bass kernel for 8 TRN2 NeuronCores.

Sharding: rows of the N x N edge grid split across 8 cores (48 rows each).
Per layer each core computes its row-block of the coord/edge MLPs and
row-sums, AllGathers the per-node x updates and msum rows; the node MLP is
computed redundantly per core. The embedding row-sum of emb_w (the dominant
HBM traffic) is sharded 1/8 per core.

Key perf structure (PE observed pinned at 1.2 GHz = 1 cyc/row for both
f32r and bf16, so coord-pass f32r precision is free):
- G=3 row groups; group g covers rows {g, 16+g, 32+g} so the three rank-2
  [w1d; A_i] matmuls land in PE row-quadrants 0/32/64 and run concurrently.
- Edge pass bf16 (halves SBUF for the m2 slab), coord pass f32r.
- Gate sigmoid via tanh (same ACT table as silu -> zero table swaps);
  sqrt via bit-trick + Newton using reciprocal_approx_fast on the DVE.
- Gated msum: gate rows partition-broadcast via DRAM stride-0 DMA, then a
  fused scalar_tensor_tensor accumulation per row (no PE, no PSUM).
- Embedding: 8 large DMAs issued up-front on 3 queues; layer-0 coordinate
  prep hoisted before the reduces to fill the DVE while DMAs stream.
"""
import os
import numpy as np

KTRUNC = int(os.environ.get("KTRUNC", "0"))

N, D, H, L = 384, 3, 128, 4
NC = 8
NI = N // NC          # 48 rows per core
NJ = N                # 384 cols
G = 3                 # rows per group (one per PE row-quadrant)
NGRP = NI // G        # 16 groups
EB = 6                # nodes per embedding DMA
NEMB = NI // EB       # 8 embedding tiles
EMB_ROWS = N * H // NC

QUAKE = 0x1FBD1DF5

_cache = {}


def _build_nc():
    import concourse.bass as bass
    import concourse.bacc as bacc
    import concourse.tile as tile
    from concourse import mybir

    F32 = mybir.dt.float32
    F32R = mybir.dt.float32r
    BF16 = mybir.dt.bfloat16
    I32 = mybir.dt.int32
    AF = mybir.ActivationFunctionType
    OP = mybir.AluOpType

    nc = bacc.Bacc(None, target_bir_lowering=False)

    def RMM(out, lhsT, rhs, **kw):
        nc.tensor.matmul(out, lhsT=lhsT.bitcast(F32R), rhs=rhs.bitcast(F32R), **kw)

    def P(name, shape, dt=F32):
        return nc.declare_dram_parameter(name, list(shape), dt, isOutput=False)

    # per-core inputs
    embw = P("embw", (EMB_ROWS, NJ))
    embbT = P("embbT", (H, NI))
    x0my = P("x0my", (NI, D))
    maskc = P("maskc", (NI, NJ))
    eyec = P("eyec", (NI, NJ))
    # shared inputs (f32)
    x0rows = P("x0rows", (1, D * NJ))
    c1hiT = P("c1hiT", (L, H, H))
    c1hjT = P("c1hjT", (L, H, H))
    cb1 = P("cb1", (H, L))
    cb2 = P("cb2", (H, L))
    c2T = P("c2T", (L, H, H))
    c3w = P("c3w", (L, H, 2 * NI - 1))
    cb3c = P("cb3c", (NI, L))
    e1hiT = P("e1hiT", (L - 1, H, H))
    eb1 = P("eb1", (H, L - 1))
    eb2 = P("eb2", (H, L - 1))
    nw1hT = P("nw1hT", (L - 1, H, H))
    nw1mT = P("nw1mT", (L - 1, H, H))
    nb1 = P("nb1", (H, L - 1))
    nw2T = P("nw2T", (L - 1, H, H))
    nb2 = P("nb2", (H, L - 1))
    cdrep = P("cdrep", (L, 16 * H))
    edrep = P("edrep", (L - 1, 16 * H))
    ones6k = P("ones6k", (1, 16 * NJ))
    # shared inputs (bf16, edge pass)
    e1hjT_b = P("e1hjT_b", (L - 1, H, H), BF16)
    e2T_b = P("e2T_b", (L - 1, H, H), BF16)
    attw_b = P("attw_b", (L - 1, H, 2 * NI - 1), BF16)

    o_x = nc.declare_dram_parameter("o_x", [N, D], F32, isOutput=True)

    # DRAM internals
    gdram = [nc.dram_tensor(f"gdram{l}", [NI * NJ], BF16) for l in range(L - 1)]
    hag_in = nc.dram_tensor("hag_in", [H, NI], F32)
    hag_out = nc.dram_tensor("hag_out", [NC * H, NI], F32, addr_space="Shared")
    xag_in = [nc.dram_tensor(f"xag_in{l}", [D, NI], F32) for l in range(L - 1)]
    xag_in.append(nc.dram_tensor("xag_in3", [NI, D], F32))
    xag_out = [nc.dram_tensor(f"xag_out{l}", [NC, D, NI], F32, addr_space="Shared")
               for l in range(L - 1)]
    xag_out.append(nc.dram_tensor("xag_out3", [N, D], F32, addr_space="Shared"))
    bar_in = nc.dram_tensor("bar_in", [1, 1], F32)
    bar_out = nc.dram_tensor("bar_out", [NC, 1], F32, addr_space="Shared")
    mag_in = [nc.dram_tensor(f"mag_in{l}", [H, NI], F32) for l in range(L - 1)]
    mag_out = [nc.dram_tensor(f"mag_out{l}", [NC * H, NI], F32, addr_space="Shared")
               for l in range(L - 1)]
    rg = [list(range(NC))]

    with tile.TileContext(nc) as tc:
        with (
            tc.tile_pool(name="consts", bufs=1) as consts,
            tc.tile_pool(name="embp", bufs=3) as embp,
            tc.tile_pool(name="cp1", bufs=1) as cp1,
            tc.tile_pool(name="cp2", bufs=2) as cp2,
            tc.tile_pool(name="work", bufs=2) as work,
            tc.tile_pool(name="slab", bufs=1) as slabp,
            tc.tile_pool(name="gbp", bufs=6) as gbp,
            tc.tile_pool(name="ps_mlp", bufs=2, space="PSUM") as ps_mlp,
            tc.tile_pool(name="ps_acc", bufs=1, space="PSUM") as ps_acc,
            tc.tile_pool(name="ps_nd", bufs=1, space="PSUM") as ps_nd,
        ):
            # ---- phase 0 ----
            # A tiny leading AllGather absorbs one-time cross-core launch
            # skew while the embedding DMAs stream on sync/scalar queues.
            nc.gpsimd.collective_compute(
                "AllGather", OP.bypass, replica_groups=rg,
                ins=[bar_in[:]], outs=[bar_out[:]])
            emb_tiles = []
            qrot = [nc.sync, nc.scalar]
            for t in range(NEMB):
                et = embp.tile([H, EB, NJ], F32, tag="embt")
                qrot[t % 2].dma_start(
                    out=et,
                    in_=embw[t * EB * H:(t + 1) * EB * H, :]
                    .rearrange("(a p) j -> p a j", p=H))
                emb_tiles.append(et)

            # ---- constants (scalar queue; ACT is idle during emb) ----
            def load(pname, ap_in, shape, dt=F32, rnd=False):
                t = consts.tile(list(shape), dt, tag=pname)
                o = t[:].bitcast(F32R) if rnd else t
                nc.scalar.dma_start(
                    out=o, in_=ap_in.bitcast(F32R) if rnd else ap_in)
                return t

            x0my_sb = load("x0my", x0my[:], (NI, D))
            embbT_sb = load("embbT", embbT[:], (H, NI))
            maskc_sb = load("maskc", maskc[:], (NI, NJ))
            eyec_sb = load("eyec", eyec[:], (NI, NJ))
            c1hiT_sb = load("c1hiT", c1hiT.rearrange("l p x -> p l x"), (H, L, H),
                            rnd=True)
            c1hjT_sb = load("c1hjT", c1hjT.rearrange("l p x -> p l x"), (H, L, H),
                            rnd=True)
            c2T_sb = load("c2T", c2T.rearrange("l p x -> p l x"), (H, L, H),
                          rnd=True)
            c3w_sb = load("c3w", c3w.rearrange("l p x -> p l x"),
                          (H, L, 2 * NI - 1), rnd=True)
            cb1_sb = load("cb1", cb1[:], (H, L))
            cb2_sb = load("cb2", cb2[:], (H, L))
            cb3c_sb = load("cb3c", cb3c[:], (NI, L))
            e1hiT_sb = load("e1hiT", e1hiT.rearrange("l p x -> p l x"),
                            (H, L - 1, H), rnd=True)
            eb1_sb = load("eb1", eb1[:], (H, L - 1))
            eb2_sb = load("eb2", eb2[:], (H, L - 1))
            nw1hT_sb = load("nw1hT", nw1hT.rearrange("l p x -> p l x"),
                            (H, L - 1, H), rnd=True)
            nw1mT_sb = load("nw1mT", nw1mT.rearrange("l p x -> p l x"),
                            (H, L - 1, H), rnd=True)
            nb1_sb = load("nb1", nb1[:], (H, L - 1))
            nw2T_sb = load("nw2T", nw2T.rearrange("l p x -> p l x"),
                           (H, L - 1, H), rnd=True)
            nb2_sb = load("nb2", nb2[:], (H, L - 1))
            cdrep_sb = load("cdrep", cdrep[:], (L, 16 * H), rnd=True)
            edrep_sb = load("edrep", edrep[:], (L - 1, 16 * H), rnd=True)
            e1hjT_w = load("e1hjT_b", e1hjT_b.rearrange("l p x -> p l x"),
                           (H, L - 1, H), BF16)
            e2T_w = load("e2T_b", e2T_b.rearrange("l p x -> p l x"),
                         (H, L - 1, H), BF16)
            attw_w = load("attw_b", attw_b.rearrange("l p x -> p l x"),
                          (H, L - 1, 2 * NI - 1), BF16)

            # d2/ones interleaved stripes (f32r): rows 32k = d2 rows
            # 16k..16k+15 flattened, rows 32k+1 = ones.
            d2ones = consts.tile([66, 16 * NJ], F32, tag="d2ones")
            combC = consts.tile([66, 16 * H], F32, tag="combC")
            combE = consts.tile([66, 16 * H], F32, tag="combE")
            for k in range(3):
                nc.scalar.dma_start(
                    out=d2ones[32 * k + 1:32 * k + 2, :].bitcast(F32R),
                    in_=ones6k[:].bitcast(F32R))

            def coord_prep(l, x_my):
                """xb loads + diff/d2/u chain + d2 stripes for layer l."""
                diff = []
                for c in range(D):
                    xb = cp1.tile([NI, NJ], F32, tag=f"xb{c}")
                    if l == 0:
                        bsrc = bass.AP(tensor=x0rows, offset=c * NJ,
                                       ap=[[0, NI], [1, NJ]])
                    else:
                        bsrc = bass.AP(tensor=xag_out[l - 1], offset=c * NI,
                                       ap=[[0, NI], [D * NI, NC], [1, NI]])
                    nc.sync.dma_start(out=xb, in_=bsrc)
                    dc = cp2.tile([NI, NJ], F32, tag=f"diff{c}")
                    nc.vector.tensor_scalar(
                        out=dc, in0=xb, scalar1=x_my[:, c:c + 1], scalar2=None,
                        op0=OP.subtract)
                    diff.append(dc)
                d2 = cp1.tile([NI, NJ], F32, tag="d2")
                tmp = cp1.tile([NI, NJ], F32, tag="ctmp")
                nc.vector.tensor_tensor(out=d2, in0=diff[0], in1=diff[0],
                                        op=OP.mult)
                nc.vector.tensor_tensor(out=tmp, in0=diff[1], in1=diff[1],
                                        op=OP.mult)
                nc.vector.tensor_tensor(out=d2, in0=d2, in1=tmp, op=OP.add)
                nc.vector.tensor_tensor(out=tmp, in0=diff[2], in1=diff[2],
                                        op=OP.mult)
                nc.vector.tensor_tensor(out=d2, in0=d2, in1=tmp, op=OP.add)
                for k in range(3):
                    nc.sync.dma_start(
                        out=d2ones[32 * k:32 * k + 1, :].bitcast(F32R),
                        in_=d2[16 * k:16 * (k + 1), :].bitcast(F32R))
                d2s = cp1.tile([NI, NJ], F32, tag="d2s")
                nc.vector.tensor_tensor(out=d2s, in0=d2, in1=eyec_sb, op=OP.add)
                # sqrt via bit-trick seed + 1 Newton step (approx recip)
                sq = cp1.tile([NI, NJ], F32, tag="sq")
                with nc.allow_low_precision(reason="bit-trick sqrt"):
                    nc.vector.tensor_scalar(
                        out=sq.bitcast(I32), in0=d2s.bitcast(I32),
                        scalar1=1, scalar2=None, op0=OP.logical_shift_right)
                    nc.vector.tensor_scalar(
                        out=sq.bitcast(I32), in0=sq.bitcast(I32),
                        scalar1=QUAKE, scalar2=None, op0=OP.add)
                nc.vector.reciprocal_approx_fast(out=tmp, in_=sq)
                nc.vector.tensor_tensor(out=tmp, in0=d2s, in1=tmp, op=OP.mult)
                nc.vector.tensor_tensor(out=sq, in0=sq, in1=tmp, op=OP.add)
                nc.vector.tensor_scalar(out=sq, in0=sq, scalar1=0.5,
                                        scalar2=None, op0=OP.mult)
                nc.vector.tensor_scalar(out=sq, in0=sq, scalar1=1.0,
                                        scalar2=None, op0=OP.add)
                u = cp2.tile([NI, NJ], F32, tag="u")
                nc.vector.reciprocal_approx_fast(out=u, in_=sq)
                nc.vector.tensor_tensor(out=u, in0=u, in1=maskc_sb, op=OP.mult)
                return diff, u

            prep0 = coord_prep(0, x0my_sb)

            # ---- embedding reduce + h AllGather ----
            hT0 = cp1.tile([H, NI], F32, tag="hT0")
            for t in range(NEMB):
                with nc.allow_low_precision(reason="f32r storage, f32 accum"):
                    nc.vector.tensor_reduce(
                        out=hT0[:, t * EB:(t + 1) * EB].bitcast(F32R),
                        in_=emb_tiles[t],
                        axis=mybir.AxisListType.X, op=OP.add)
            nc.vector.tensor_tensor(out=hT0[:].bitcast(F32R), in0=hT0,
                                    in1=embbT_sb, op=OP.add)
            nc.gpsimd.dma_start(out=hag_in[:], in_=hT0)
            nc.gpsimd.collective_compute(
                "AllGather", OP.bypass, replica_groups=rg,
                ins=[hag_in[:]], outs=[hag_out[:]])
            h_T = cp2.tile([H, NJ], F32, tag="hT")
            for r in range(NC):
                nc.sync.dma_start(
                    out=h_T[:, r * NI:(r + 1) * NI].bitcast(F32R),
                    in_=hag_out[r * H:(r + 1) * H, :].bitcast(F32R))
            h_my = hT0
            hT_bf = cp2.tile([H, NJ], BF16, tag="hTb")
            nc.vector.tensor_copy(hT_bf, h_T)

            x_my = x0my_sb

            for l in range(L):
                if KTRUNC == 1 or (KTRUNC and l > 0):
                    break
                last = l == L - 1
                do_gate = (not last) and KTRUNC in (0, 3, 4)
                do_coord = KTRUNC in (0, 3, 4)
                do_node = (not last) and KTRUNC in (0, 4)

                diff, u = prep0 if l == 0 else coord_prep(l, x_my)

                # A-terms: At[i, :] = (W1hi @ h_i), striped into comb pairs
                def make_comb(hiT, wdrep_row, comb, atag):
                    At_ps = ps_nd.tile([NI, H], F32, tag="nd")
                    RMM(At_ps, lhsT=h_my, rhs=hiT, start=True, stop=True)
                    At = cp1.tile([NI, H], F32, tag=atag)
                    nc.vector.tensor_copy(At[:].bitcast(F32R), At_ps)
                    for k in range(3):
                        nc.sync.dma_start(
                            out=comb[32 * k:32 * k + 1, :].bitcast(F32R),
                            in_=wdrep_row.bitcast(F32R))
                        nc.sync.dma_start(
                            out=comb[32 * k + 1:32 * k + 2, :].bitcast(F32R),
                            in_=At[16 * k:16 * (k + 1), :].bitcast(F32R))

                make_comb(c1hiT_sb[:, l, :], cdrep_sb[l:l + 1, :], combC, "Atc")
                if not last:
                    make_comb(e1hiT_sb[:, l, :], edrep_sb[l:l + 1, :], combE,
                              "Ate")

                # group g covers rows {g, 16+g, 32+g}: the three rank-2
                # matmuls hit PE row-quadrants 0/32/64 and run concurrently
                def mm_d(pre, comb, g):
                    for r in range(G):
                        nc.tensor.matmul(
                            pre[:, r * 512:r * 512 + NJ],
                            lhsT=comb[32 * r:32 * r + 2,
                                      g * H:(g + 1) * H].bitcast(F32R),
                            rhs=d2ones[32 * r:32 * r + 2,
                                       g * NJ:(g + 1) * NJ].bitcast(F32R),
                            start=False, stop=True)

                # ---- edge pass (bf16) ----
                if not last:
                    att_ps = ps_acc.tile([H, NJ], F32, tag="acc")
                    m2slab = slabp.tile([H, NI * NJ], BF16, tag="m2")
                    for g in range(NGRP):
                        pre = ps_mlp.tile([H, G * 512], F32, tag="mlp")
                        for r in range(G):
                            nc.tensor.matmul(
                                pre[:, r * 512:r * 512 + NJ],
                                lhsT=e1hjT_w[:, l, :], rhs=hT_bf,
                                start=True, stop=False)
                        mm_d(pre, combE, g)
                        t1 = work.tile([H, G * NJ], BF16, tag="t1")
                        nc.scalar.activation(
                            out=t1.rearrange("p (r c) -> p r c", r=G),
                            in_=pre.rearrange("p (r c) -> p r c", r=G)[:, :, 0:NJ],
                            func=AF.Silu, bias=eb1_sb[:, l:l + 1], scale=1.0)
                        z2 = ps_mlp.tile([H, G * 512], F32, tag="mlp")
                        nc.tensor.matmul(z2[:, 0:512], lhsT=e2T_w[:, l, :],
                                         rhs=t1[:, 0:512], start=True, stop=True)
                        nc.tensor.matmul(z2[:, 512:1024], lhsT=e2T_w[:, l, :],
                                         rhs=t1[:, 512:1024], start=True,
                                         stop=True)
                        nc.tensor.matmul(z2[:, 1024:1152], lhsT=e2T_w[:, l, :],
                                         rhs=t1[:, 1024:1152], start=True,
                                         stop=True)
                        nc.scalar.activation(
                            out=m2slab[:, g * G * NJ:(g + 1) * G * NJ],
                            in_=z2[:, 0:G * NJ],
                            func=AF.Silu, bias=eb2_sb[:, l:l + 1], scale=1.0)
                        for r in range(G):
                            i = 16 * r + g
                            s = G * g + r
                            nc.tensor.matmul(
                                att_ps[0:NI, :],
                                lhsT=attw_w[:, l, (NI - 1) - i:(2 * NI - 1) - i],
                                rhs=m2slab[:, s * NJ:(s + 1) * NJ],
                                start=(s == 0), stop=(s == NI - 1))

                    # gate: exact sigmoid via tanh (same ACT table as silu)
                    if not do_gate:
                        continue
                    sg = cp1.tile([NI, NJ], F32, tag="sg")
                    nc.scalar.activation(out=sg, in_=att_ps[0:NI, :],
                                         func=AF.Tanh, bias=0.0, scale=0.5)
                    gmask = cp1.tile([NI, NJ], F32, tag="gmask")
                    nc.vector.tensor_scalar(out=gmask, in0=sg, scalar1=0.5,
                                            scalar2=0.5, op0=OP.mult,
                                            op1=OP.add)
                    nc.vector.tensor_tensor(out=gmask, in0=gmask, in1=maskc_sb,
                                            op=OP.mult)
                    gmb = cp1.tile([NI, NJ], BF16, tag="gmb")
                    nc.vector.tensor_copy(gmb, gmask)
                    nc.sync.dma_start(out=gdram[l][:], in_=gmb)
                    # partition-broadcast gate rows via DRAM stride-0 reads,
                    # in slab-slot order s = G*g + r  ->  grid row 16*r + g
                    gb_tiles = [None] * NI
                    for g in range(NGRP):
                        for r in range(G):
                            i = 16 * r + g
                            s = G * g + r
                            gb = gbp.tile([H, NJ], BF16, tag="gb")
                            eng = nc.sync if s % 2 == 0 else nc.gpsimd
                            eng.dma_start(
                                out=gb,
                                in_=bass.AP(tensor=gdram[l], offset=i * NJ,
                                            ap=[[0, H], [1, NJ]]))
                            gb_tiles[s] = gb
                    msumT = cp1.tile([H, NI], F32, tag="msumT")
                    mgs = cp1.tile([H, NJ], BF16, tag="mgs")

                # ---- coord pass (f32r) ----
                if do_coord:
                    phi_ps = ps_acc.tile([H, NJ], F32, tag="acc")

                    def msum_row(s):
                        i = 16 * (s % G) + s // G
                        nc.vector.scalar_tensor_tensor(
                            out=mgs, in0=m2slab[:, s * NJ:(s + 1) * NJ],
                            scalar=1.0, in1=gb_tiles[s],
                            op0=OP.mult, op1=OP.mult,
                            accum_out=msumT[:, i:i + 1])

                    for g in range(NGRP):
                        pre = ps_mlp.tile([H, G * 512], F32, tag="mlp")
                        for r in range(G):
                            RMM(pre[:, r * 512:r * 512 + NJ],
                                lhsT=c1hjT_sb[:, l, :], rhs=h_T,
                                start=True, stop=False)
                        mm_d(pre, combC, g)
                        t1c = work.tile([H, G * NJ], F32, tag="t1c")
                        nc.scalar.activation(
                            out=t1c[:].bitcast(F32R)
                            .rearrange("p (r c) -> p r c", r=G),
                            in_=pre.rearrange("p (r c) -> p r c", r=G)[:, :, 0:NJ],
                            func=AF.Silu, bias=cb1_sb[:, l:l + 1], scale=1.0)
                        z2 = ps_mlp.tile([H, G * 512], F32, tag="mlp")
                        RMM(z2[:, 0:512], lhsT=c2T_sb[:, l, :],
                            rhs=t1c[:, 0:512], start=True, stop=True)
                        RMM(z2[:, 512:1024], lhsT=c2T_sb[:, l, :],
                            rhs=t1c[:, 512:1024], start=True, stop=True)
                        RMM(z2[:, 1024:1152], lhsT=c2T_sb[:, l, :],
                            rhs=t1c[:, 1024:1152], start=True, stop=True)
                        t2c = work.tile([H, G * NJ], F32, tag="t2c")
                        nc.scalar.activation(
                            out=t2c[:].bitcast(F32R),
                            in_=z2[:, 0:G * NJ],
                            func=AF.Silu, bias=cb2_sb[:, l:l + 1], scale=1.0)
                        for r in range(G):
                            i = 16 * r + g
                            s = G * g + r
                            RMM(phi_ps[0:NI, :],
                                lhsT=c3w_sb[:, l,
                                            (NI - 1) - i:(2 * NI - 1) - i],
                                rhs=t2c[:, r * NJ:(r + 1) * NJ],
                                start=(s == 0), stop=(s == NI - 1))
                        if do_gate:
                            for r in range(G):
                                msum_row(G * g + r)

                if do_node:
                    # msum AllGather + node MLP (overlaps late coord groups)
                    nc.gpsimd.dma_start(out=mag_in[l][:], in_=msumT)
                    nc.gpsimd.collective_compute(
                        "AllGather", OP.bypass, replica_groups=rg,
                        ins=[mag_in[l][:]], outs=[mag_out[l][:]])
                    msumF = cp1.tile([H, NJ], F32, tag="msumF")
                    for r in range(NC):
                        nc.gpsimd.dma_start(
                            out=msumF[:, r * NI:(r + 1) * NI].bitcast(F32R),
                            in_=mag_out[l][r * H:(r + 1) * H, :].bitcast(F32R))
                    z1 = ps_nd.tile([H, 512], F32, tag="nd")
                    RMM(z1[:, 0:NJ], lhsT=nw1hT_sb[:, l, :], rhs=h_T,
                        start=True, stop=False)
                    RMM(z1[:, 0:NJ], lhsT=nw1mT_sb[:, l, :], rhs=msumF,
                        start=False, stop=True)
                    t1n = cp1.tile([H, NJ], F32, tag="t1n")
                    nc.scalar.activation(out=t1n[:].bitcast(F32R),
                                         in_=z1[:, 0:NJ], func=AF.Silu,
                                         bias=nb1_sb[:, l:l + 1], scale=1.0)
                    z2n = ps_nd.tile([H, 512], F32, tag="nd")
                    RMM(z2n[:, 0:NJ], lhsT=nw2T_sb[:, l, :], rhs=t1n,
                        start=True, stop=True)
                    h_T = cp2.tile([H, NJ], F32, tag="hT")
                    nc.vector.tensor_scalar(out=h_T[:].bitcast(F32R),
                                            in0=z2n[:, 0:NJ],
                                            scalar1=nb2_sb[:, l:l + 1],
                                            scalar2=None, op0=OP.add)
                    if l < L - 2:
                        hT_bf = cp2.tile([H, NJ], BF16, tag="hTb")
                        nc.vector.tensor_copy(hT_bf, h_T)
                    # local copy of this core's own h rows
                    z1m = ps_nd.tile([H, 512], F32, tag="nd")
                    nc.tensor.matmul(z1m[:, 0:NI], lhsT=nw1hT_sb[:, l, :],
                                     rhs=h_my, start=True, stop=False)
                    nc.tensor.matmul(z1m[:, 0:NI], lhsT=nw1mT_sb[:, l, :],
                                     rhs=msumT, start=False, stop=True)
                    t1m = cp1.tile([H, NI], F32, tag="t1m")
                    nc.scalar.activation(out=t1m, in_=z1m[:, 0:NI],
                                         func=AF.Silu,
                                         bias=nb1_sb[:, l:l + 1], scale=1.0)
                    z2m = ps_nd.tile([H, 512], F32, tag="nd")
                    nc.tensor.matmul(z2m[:, 0:NI], lhsT=nw2T_sb[:, l, :],
                                     rhs=t1m, start=True, stop=True)
                    h_my = cp2.tile([H, NI], F32, tag="hmy")
                    nc.vector.tensor_scalar(out=h_my[:].bitcast(F32R),
                                            in0=z2m[:, 0:NI],
                                            scalar1=nb2_sb[:, l:l + 1],
                                            scalar2=None, op0=OP.add)

                if do_coord:
                    # ---- phi stream + x update ----
                    phis = cp1.tile([NI, NJ], F32, tag="phis")
                    nc.vector.tensor_scalar(out=phis, in0=phi_ps[0:NI, :],
                                            scalar1=cb3c_sb[:, l:l + 1],
                                            scalar2=None, op0=OP.add)
                    s = cp1.tile([NI, NJ], F32, tag="s")
                    nc.vector.tensor_tensor(out=s, in0=phis, in1=u, op=OP.mult)
                    xnew = cp2.tile([NI, D], F32, tag="xnew")
                    xms = cp1.tile([NI, NJ], F32, tag="xms")
                    for c in range(D):
                        xcol = cp1.tile([NI, 1], F32, tag=f"xcol{c}")
                        nc.vector.scalar_tensor_tensor(
                            out=xms, in0=diff[c], scalar=1.0, in1=s,
                            op0=OP.mult, op1=OP.mult, accum_out=xcol)
                        nc.vector.tensor_tensor(
                            out=xnew[:, c:c + 1], in0=xcol,
                            in1=x_my[:, c:c + 1], op=OP.add)
                    if not last:
                        nc.gpsimd.dma_start(
                            out=xag_in[l].rearrange("c n -> n c"), in_=xnew)
                    else:
                        nc.gpsimd.dma_start(out=xag_in[l][:], in_=xnew)
                    nc.gpsimd.collective_compute(
                        "AllGather", OP.bypass, replica_groups=rg,
                        ins=[xag_in[l][:]], outs=[xag_out[l][:]])
                    if not last:
                        x_my = xnew
                    else:
                        nc.sync.dma_start(out=o_x[:], in_=xag_out[l][:])

            if KTRUNC:
                nc.sync.dma_start(out=o_x[0:NI, :], in_=x0my_sb)

    nc.finalize()
    return nc


def _prep_inputs(inputs):
    import ml_dtypes
    BF = ml_dtypes.bfloat16
    f = lambda a: np.ascontiguousarray(np.asarray(a), dtype=np.float32)
    b = lambda a: np.ascontiguousarray(np.asarray(a, dtype=np.float32)
                                       .astype(BF))
    x_inp = f(inputs["x_inp"])
    emb_w = f(inputs["emb_w"])
    emb_b = f(inputs["emb_b"])
    coord_w1 = f(inputs["coord_w1"])
    coord_b1 = f(inputs["coord_b1"])
    coord_w2 = f(inputs["coord_w2"])
    coord_b2 = f(inputs["coord_b2"])
    coord_w3 = f(inputs["coord_w3"])
    coord_b3 = f(inputs["coord_b3"])
    edge_w1 = f(inputs["edge_w1"])
    edge_b1 = f(inputs["edge_b1"])
    edge_w2 = f(inputs["edge_w2"])
    edge_b2 = f(inputs["edge_b2"])
    node_w1 = f(inputs["node_w1"])
    node_b1 = f(inputs["node_b1"])
    node_w2 = f(inputs["node_w2"])
    node_b2 = f(inputs["node_b2"])
    att_w = f(inputs["att_w"])

    x0 = x_inp.reshape(N, D)
    eye = np.eye(N, dtype=np.float32)

    def stackT(w, lo, hi):
        return np.ascontiguousarray(
            np.stack([w[l, :, lo:hi].T for l in range(w.shape[0])]))

    def win(w3):
        nl = w3.shape[0]
        out = np.zeros((nl, H, 2 * NI - 1), np.float32)
        out[:, :, NI - 1] = w3[:, 0, :]
        return out

    shared = dict(
        x0rows=np.ascontiguousarray(x0.T.reshape(1, D * N)),
        c1hiT=stackT(coord_w1, 0, H),
        c1hjT=stackT(coord_w1, H, 2 * H),
        cb1=np.ascontiguousarray(coord_b1.T),
        cb2=np.ascontiguousarray(coord_b2.T),
        c2T=np.ascontiguousarray(np.stack([coord_w2[l].T for l in range(L)])),
        c3w=win(coord_w3),
        cb3c=np.ascontiguousarray(
            np.broadcast_to(coord_b3[:, 0][None, :], (NI, L))),
        e1hiT=stackT(edge_w1, 0, H),
        eb1=np.ascontiguousarray(edge_b1.T),
        eb2=np.ascontiguousarray(edge_b2.T),
        nw1hT=stackT(node_w1, 0, H),
        nw1mT=stackT(node_w1, H, 2 * H),
        nb1=np.ascontiguousarray(node_b1.T),
        nw2T=np.ascontiguousarray(np.stack([node_w2[l].T for l in range(L - 1)])),
        nb2=np.ascontiguousarray(node_b2.T),
        cdrep=np.ascontiguousarray(np.tile(coord_w1[:, :, 2 * H], (1, 16))),
        edrep=np.ascontiguousarray(np.tile(edge_w1[:, :, 2 * H], (1, 16))),
        ones6k=np.ones((1, 16 * NJ), np.float32),
        e1hjT_b=b(stackT(edge_w1, H, 2 * H)),
        e2T_b=b(np.stack([edge_w2[l].T for l in range(L - 1)])),
        attw_b=b(win(att_w)),
    )
    in_maps = []
    for c in range(NC):
        m = dict(shared)
        m["embw"] = np.ascontiguousarray(
            emb_w[c * EMB_ROWS:(c + 1) * EMB_ROWS, :])
        m["embbT"] = np.ascontiguousarray(
            emb_b[c * EMB_ROWS:(c + 1) * EMB_ROWS].reshape(NI, H).T)
        m["x0my"] = np.ascontiguousarray(x0[c * NI:(c + 1) * NI, :])
        m["maskc"] = np.ascontiguousarray(1.0 - eye[c * NI:(c + 1) * NI, :])
        m["eyec"] = np.ascontiguousarray(eye[c * NI:(c + 1) * NI, :])
        in_maps.append(m)
    return in_maps


def _run(inputs, trace=False, **kw):
    from concourse.bass_utils import run_bass_kernel_spmd
    if "nc" not in _cache:
        _cache["nc"] = _build_nc()
    in_maps = _prep_inputs(inputs)
    return run_bass_kernel_spmd(_cache["nc"], in_maps, list(range(NC)),
                                trace=trace, **kw)


def kernel(**inputs) -> np.ndarray:
    res = _run(inputs)
    return np.asarray(res.results[0]["o_x"], dtype=np.float32).reshape(N * D)


# revision 15
# speedup vs baseline: 1.4898x; 1.0245x over previous
"""EGNN (N=384, D=3, H=128, L=4) Bass kernel for 8 TRN2 NeuronCores.

Sharding: rows of the N x N edge grid split across 8 cores (48 rows each).
Per layer each core computes its row-block of the coord/edge MLPs and
row-sums, AllGathers the per-node x updates and msum rows; the node MLP is
computed redundantly per core. The embedding row-sum of emb_w (the dominant
HBM traffic) is sharded 1/8 per core.

Key perf structure (PE observed pinned at 1.2 GHz = 1 cyc/row for both
f32r and bf16, so coord-pass f32r precision is free):
- G=3 row groups; group g covers rows {g, 16+g, 32+g} so the three rank-2
  [w1d; A_i] matmuls land in PE row-quadrants 0/32/64 and run concurrently.
- Edge pass bf16 (halves SBUF for the m2 slab), coord pass f32r.
- Gate sigmoid via tanh (same ACT table as silu -> zero table swaps);
  sqrt via bit-trick + Newton using reciprocal_approx_fast on the DVE.
- Gated msum: gate rows partition-broadcast via DRAM stride-0 DMA, then a
  fused scalar_tensor_tensor accumulation per row (no PE, no PSUM).
- Embedding: 8 large DMAs issued up-front on 3 queues; layer-0 coordinate
  prep hoisted before the reduces to fill the DVE while DMAs stream.
"""
import os
import numpy as np

KTRUNC = int(os.environ.get("KTRUNC", "0"))

N, D, H, L = 384, 3, 128, 4
NC = 8
NI = N // NC          # 48 rows per core
NJ = N                # 384 cols
G = 3                 # rows per group (one per PE row-quadrant)
NGRP = NI // G        # 16 groups
EB = 6                # nodes per embedding DMA
NEMB = NI // EB       # 8 embedding tiles
EMB_ROWS = N * H // NC

QUAKE = 0x1FBD1DF5

_cache = {}


def _build_nc():
    import concourse.bass as bass
    import concourse.bacc as bacc
    import concourse.tile as tile
    from concourse import mybir

    F32 = mybir.dt.float32
    F32R = mybir.dt.float32r
    BF16 = mybir.dt.bfloat16
    I32 = mybir.dt.int32
    AF = mybir.ActivationFunctionType
    OP = mybir.AluOpType

    nc = bacc.Bacc(None, target_bir_lowering=False)

    def RMM(out, lhsT, rhs, **kw):
        nc.tensor.matmul(out, lhsT=lhsT.bitcast(F32R), rhs=rhs.bitcast(F32R), **kw)

    def P(name, shape, dt=F32):
        return nc.declare_dram_parameter(name, list(shape), dt, isOutput=False)

    # per-core inputs
    embw = P("embw", (EMB_ROWS, NJ))
    embbT = P("embbT", (H, NI))
    x0my = P("x0my", (NI, D))
    maskc = P("maskc", (NI, NJ))
    eyec = P("eyec", (NI, NJ))
    # shared inputs (f32)
    x0rows = P("x0rows", (1, D * NJ))
    c1hiT = P("c1hiT", (L, H, H))
    c1hjT = P("c1hjT", (L, H, H))
    cb1 = P("cb1", (H, L))
    cb2 = P("cb2", (H, L))
    c2T = P("c2T", (L, H, H))
    c3w = P("c3w", (L, H, 2 * NI - 1))
    cb3c = P("cb3c", (NI, L))
    e1hiT = P("e1hiT", (L - 1, H, H))
    eb1 = P("eb1", (H, L - 1))
    eb2 = P("eb2", (H, L - 1))
    nw1hT = P("nw1hT", (L - 1, H, H))
    nw1mT = P("nw1mT", (L - 1, H, H))
    nb1 = P("nb1", (H, L - 1))
    nw2T = P("nw2T", (L - 1, H, H))
    nb2 = P("nb2", (H, L - 1))
    cdrep = P("cdrep", (L, 16 * H))
    edrep = P("edrep", (L - 1, 16 * H))
    ones6k = P("ones6k", (1, 16 * NJ))
    # shared inputs (bf16, edge pass)
    e1hjT_b = P("e1hjT_b", (L - 1, H, H), BF16)
    e2T_b = P("e2T_b", (L - 1, H, H), BF16)
    attw_b = P("attw_b", (L - 1, H, 2 * NI - 1), BF16)

    o_x = nc.declare_dram_parameter("o_x", [N, D], F32, isOutput=True)

    # DRAM internals
    gdram = [nc.dram_tensor(f"gdram{l}", [NI * NJ], BF16) for l in range(L - 1)]
    hag_in = nc.dram_tensor("hag_in", [H, NI], F32)
    hag_out = nc.dram_tensor("hag_out", [NC * H, NI], F32, addr_space="Shared")
    xag_in = [nc.dram_tensor(f"xag_in{l}", [D, NI], F32) for l in range(L - 1)]
    xag_in.append(nc.dram_tensor("xag_in3", [NI, D], F32))
    xag_out = [nc.dram_tensor(f"xag_out{l}", [NC, D, NI], F32, addr_space="Shared")
               for l in range(L - 1)]
    xag_out.append(nc.dram_tensor("xag_out3", [N, D], F32, addr_space="Shared"))
    bar_in = nc.dram_tensor("bar_in", [1, 1], F32)
    bar_out = nc.dram_tensor("bar_out", [NC, 1], F32, addr_space="Shared")
    mag_in = [nc.dram_tensor(f"mag_in{l}", [H, NI], F32) for l in range(L - 1)]
    mag_out = [nc.dram_tensor(f"mag_out{l}", [NC * H, NI], F32, addr_space="Shared")
               for l in range(L - 1)]
    rg = [list(range(NC))]

    with tile.TileContext(nc) as tc:
        with (
            tc.tile_pool(name="consts", bufs=1) as consts,
            tc.tile_pool(name="embp", bufs=3) as embp,
            tc.tile_pool(name="cp1", bufs=1) as cp1,
            tc.tile_pool(name="cp2", bufs=2) as cp2,
            tc.tile_pool(name="work", bufs=2) as work,
            tc.tile_pool(name="slab", bufs=1) as slabp,
            tc.tile_pool(name="gbp", bufs=6) as gbp,
            tc.tile_pool(name="ps_mlp", bufs=2, space="PSUM") as ps_mlp,
            tc.tile_pool(name="ps_acc", bufs=1, space="PSUM") as ps_acc,
            tc.tile_pool(name="ps_nd", bufs=1, space="PSUM") as ps_nd,
        ):
            # ---- phase 0 ----
            # A tiny leading AllGather absorbs one-time cross-core launch
            # skew while the embedding DMAs stream on sync/scalar queues.
            nc.gpsimd.collective_compute(
                "AllGather", OP.bypass, replica_groups=rg,
                ins=[bar_in[:]], outs=[bar_out[:]])
            emb_tiles = []
            qrot = [nc.sync, nc.scalar]
            for t in range(NEMB):
                et = embp.tile([H, EB, NJ], F32, tag="embt")
                qrot[t % 2].dma_start(
                    out=et,
                    in_=embw[t * EB * H:(t + 1) * EB * H, :]
                    .rearrange("(a p) j -> p a j", p=H))
                emb_tiles.append(et)

            # ---- constants (scalar queue; ACT is idle during emb) ----
            def load(pname, ap_in, shape, dt=F32, rnd=False):
                t = consts.tile(list(shape), dt, tag=pname)
                o = t[:].bitcast(F32R) if rnd else t
                nc.scalar.dma_start(
                    out=o, in_=ap_in.bitcast(F32R) if rnd else ap_in)
                return t

            x0my_sb = load("x0my", x0my[:], (NI, D))
            embbT_sb = load("embbT", embbT[:], (H, NI))
            maskc_sb = load("maskc", maskc[:], (NI, NJ))
            eyec_sb = load("eyec", eyec[:], (NI, NJ))
            c1hiT_sb = load("c1hiT", c1hiT.rearrange("l p x -> p l x"), (H, L, H),
                            rnd=True)
            c1hjT_sb = load("c1hjT", c1hjT.rearrange("l p x -> p l x"), (H, L, H),
                            rnd=True)
            c2T_sb = load("c2T", c2T.rearrange("l p x -> p l x"), (H, L, H),
                          rnd=True)
            c3w_sb = load("c3w", c3w.rearrange("l p x -> p l x"),
                          (H, L, 2 * NI - 1), rnd=True)
            cb1_sb = load("cb1", cb1[:], (H, L))
            cb2_sb = load("cb2", cb2[:], (H, L))
            cb3c_sb = load("cb3c", cb3c[:], (NI, L))
            e1hiT_sb = load("e1hiT", e1hiT.rearrange("l p x -> p l x"),
                            (H, L - 1, H), rnd=True)
            eb1_sb = load("eb1", eb1[:], (H, L - 1))
            eb2_sb = load("eb2", eb2[:], (H, L - 1))
            nw1hT_sb = load("nw1hT", nw1hT.rearrange("l p x -> p l x"),
                            (H, L - 1, H), rnd=True)
            nw1mT_sb = load("nw1mT", nw1mT.rearrange("l p x -> p l x"),
                            (H, L - 1, H), rnd=True)
            nb1_sb = load("nb1", nb1[:], (H, L - 1))
            nw2T_sb = load("nw2T", nw2T.rearrange("l p x -> p l x"),
                           (H, L - 1, H), rnd=True)
            nb2_sb = load("nb2", nb2[:], (H, L - 1))
            cdrep_sb = load("cdrep", cdrep[:], (L, 16 * H), rnd=True)
            edrep_sb = load("edrep", edrep[:], (L - 1, 16 * H), rnd=True)
            e1hjT_w = load("e1hjT_b", e1hjT_b.rearrange("l p x -> p l x"),
                           (H, L - 1, H), BF16)
            e2T_w = load("e2T_b", e2T_b.rearrange("l p x -> p l x"),
                         (H, L - 1, H), BF16)
            attw_w = load("attw_b", attw_b.rearrange("l p x -> p l x"),
                          (H, L - 1, 2 * NI - 1), BF16)

            # d2/ones interleaved stripes (f32r): rows 32k = d2 rows
            # 16k..16k+15 flattened, rows 32k+1 = ones.
            d2ones = consts.tile([66, 16 * NJ], F32, tag="d2ones")
            combC = consts.tile([66, 16 * H], F32, tag="combC")
            combE = consts.tile([66, 16 * H], F32, tag="combE")
            for k in range(3):
                nc.scalar.dma_start(
                    out=d2ones[32 * k + 1:32 * k + 2, :].bitcast(F32R),
                    in_=ones6k[:].bitcast(F32R))

            def coord_prep(l, x_my):
                """xb loads + diff/d2/u chain + d2 stripes for layer l."""
                diff = []
                for c in range(D):
                    xb = cp1.tile([NI, NJ], F32, tag=f"xb{c}")
                    if l == 0:
                        bsrc = bass.AP(tensor=x0rows, offset=c * NJ,
                                       ap=[[0, NI], [1, NJ]])
                    else:
                        bsrc = bass.AP(tensor=xag_out[l - 1], offset=c * NI,
                                       ap=[[0, NI], [D * NI, NC], [1, NI]])
                    nc.sync.dma_start(out=xb, in_=bsrc)
                    dc = cp2.tile([NI, NJ], F32, tag=f"diff{c}")
                    nc.vector.tensor_scalar(
                        out=dc, in0=xb, scalar1=x_my[:, c:c + 1], scalar2=None,
                        op0=OP.subtract)
                    diff.append(dc)
                d2 = cp1.tile([NI, NJ], F32, tag="d2")
                tmp = cp1.tile([NI, NJ], F32, tag="ctmp")
                nc.vector.tensor_tensor(out=d2, in0=diff[0], in1=diff[0],
                                        op=OP.mult)
                nc.vector.tensor_tensor(out=tmp, in0=diff[1], in1=diff[1],
                                        op=OP.mult)
                nc.vector.tensor_tensor(out=d2, in0=d2, in1=tmp, op=OP.add)
                nc.vector.tensor_tensor(out=tmp, in0=diff[2], in1=diff[2],
                                        op=OP.mult)
                nc.vector.tensor_tensor(out=d2, in0=d2, in1=tmp, op=OP.add)
                for k in range(3):
                    nc.sync.dma_start(
                        out=d2ones[32 * k:32 * k + 1, :].bitcast(F32R),
                        in_=d2[16 * k:16 * (k + 1), :].bitcast(F32R))
                d2s = cp1.tile([NI, NJ], F32, tag="d2s")
                nc.vector.tensor_tensor(out=d2s, in0=d2, in1=eyec_sb, op=OP.add)
                # sqrt via bit-trick seed + 1 Newton step (approx recip)
                sq = cp1.tile([NI, NJ], F32, tag="sq")
                with nc.allow_low_precision(reason="bit-trick sqrt"):
                    nc.vector.tensor_scalar(
                        out=sq.bitcast(I32), in0=d2s.bitcast(I32),
                        scalar1=1, scalar2=None, op0=OP.logical_shift_right)
                    nc.vector.tensor_scalar(
                        out=sq.bitcast(I32), in0=sq.bitcast(I32),
                        scalar1=QUAKE, scalar2=None, op0=OP.add)
                nc.vector.reciprocal_approx_fast(out=tmp, in_=sq)
                nc.vector.tensor_tensor(out=tmp, in0=d2s, in1=tmp, op=OP.mult)
                nc.vector.tensor_tensor(out=sq, in0=sq, in1=tmp, op=OP.add)
                nc.vector.tensor_scalar(out=sq, in0=sq, scalar1=0.5,
                                        scalar2=None, op0=OP.mult)
                nc.vector.tensor_scalar(out=sq, in0=sq, scalar1=1.0,
                                        scalar2=None, op0=OP.add)
                u = cp2.tile([NI, NJ], F32, tag="u")
                nc.vector.reciprocal_approx_fast(out=u, in_=sq)
                nc.vector.tensor_tensor(out=u, in0=u, in1=maskc_sb, op=OP.mult)
                return diff, u

            prep0 = coord_prep(0, x0my_sb)

            # ---- embedding reduce + h AllGather ----
            hT0 = cp1.tile([H, NI], F32, tag="hT0")
            for t in range(NEMB):
                with nc.allow_low_precision(reason="f32r storage, f32 accum"):
                    nc.vector.tensor_reduce(
                        out=hT0[:, t * EB:(t + 1) * EB].bitcast(F32R),
                        in_=emb_tiles[t],
                        axis=mybir.AxisListType.X, op=OP.add)
            nc.vector.tensor_tensor(out=hT0[:].bitcast(F32R), in0=hT0,
                                    in1=embbT_sb, op=OP.add)
            nc.gpsimd.dma_start(out=hag_in[:], in_=hT0)
            nc.gpsimd.collective_compute(
                "AllGather", OP.bypass, replica_groups=rg,
                ins=[hag_in[:]], outs=[hag_out[:]])
            h_T = cp2.tile([H, NJ], F32, tag="hT")
            nc.scalar.dma_start(
                out=h_T[:].bitcast(F32R).rearrange("p (r n) -> p r n", r=NC),
                in_=hag_out.rearrange("(r p) n -> p r n", p=H).bitcast(F32R))
            h_my = hT0
            hT_bf = cp2.tile([H, NJ], BF16, tag="hTb")
            nc.vector.tensor_copy(hT_bf, h_T)

            x_my = x0my_sb

            for l in range(L):
                if KTRUNC == 1 or (KTRUNC and l > 0):
                    break
                last = l == L - 1
                do_gate = (not last) and KTRUNC in (0, 3, 4)
                do_coord = KTRUNC in (0, 3, 4)
                do_node = (not last) and KTRUNC in (0, 4)

                diff, u = prep0 if l == 0 else coord_prep(l, x_my)

                # A-terms: At[i, :] = (W1hi @ h_i), striped into comb pairs
                def make_comb(hiT, wdrep_row, comb, atag):
                    At_ps = ps_nd.tile([NI, H], F32, tag="nd")
                    RMM(At_ps, lhsT=h_my, rhs=hiT, start=True, stop=True)
                    At = cp1.tile([NI, H], F32, tag=atag)
                    nc.vector.tensor_copy(At[:].bitcast(F32R), At_ps)
                    for k in range(3):
                        nc.sync.dma_start(
                            out=comb[32 * k:32 * k + 1, :].bitcast(F32R),
                            in_=wdrep_row.bitcast(F32R))
                        nc.sync.dma_start(
                            out=comb[32 * k + 1:32 * k + 2, :].bitcast(F32R),
                            in_=At[16 * k:16 * (k + 1), :].bitcast(F32R))

                make_comb(c1hiT_sb[:, l, :], cdrep_sb[l:l + 1, :], combC, "Atc")
                if not last:
                    make_comb(e1hiT_sb[:, l, :], edrep_sb[l:l + 1, :], combE,
                              "Ate")

                # group g covers rows {g, 16+g, 32+g}: the three rank-2
                # matmuls hit PE row-quadrants 0/32/64 and run concurrently
                def mm_d(pre, comb, g):
                    for r in range(G):
                        nc.tensor.matmul(
                            pre[:, r * 512:r * 512 + NJ],
                            lhsT=comb[32 * r:32 * r + 2,
                                      g * H:(g + 1) * H].bitcast(F32R),
                            rhs=d2ones[32 * r:32 * r + 2,
                                       g * NJ:(g + 1) * NJ].bitcast(F32R),
                            start=False, stop=True)

                # ---- edge pass (bf16) ----
                if not last:
                    att_ps = ps_acc.tile([H, NJ], F32, tag="acc")
                    m2slab = slabp.tile([H, NI * NJ], BF16, tag="m2")
                    for g in range(NGRP):
                        pre = ps_mlp.tile([H, G * 512], F32, tag="mlp")
                        for r in range(G):
                            nc.tensor.matmul(
                                pre[:, r * 512:r * 512 + NJ],
                                lhsT=e1hjT_w[:, l, :], rhs=hT_bf,
                                start=True, stop=False)
                        mm_d(pre, combE, g)
                        t1 = work.tile([H, G * NJ], BF16, tag="t1")
                        nc.scalar.activation(
                            out=t1.rearrange("p (r c) -> p r c", r=G),
                            in_=pre.rearrange("p (r c) -> p r c", r=G)[:, :, 0:NJ],
                            func=AF.Silu, bias=eb1_sb[:, l:l + 1], scale=1.0)
                        z2 = ps_mlp.tile([H, G * 512], F32, tag="mlp")
                        nc.tensor.matmul(z2[:, 0:512], lhsT=e2T_w[:, l, :],
                                         rhs=t1[:, 0:512], start=True, stop=True)
                        nc.tensor.matmul(z2[:, 512:1024], lhsT=e2T_w[:, l, :],
                                         rhs=t1[:, 512:1024], start=True,
                                         stop=True)
                        nc.tensor.matmul(z2[:, 1024:1152], lhsT=e2T_w[:, l, :],
                                         rhs=t1[:, 1024:1152], start=True,
                                         stop=True)
                        nc.scalar.activation(
                            out=m2slab[:, g * G * NJ:(g + 1) * G * NJ],
                            in_=z2[:, 0:G * NJ],
                            func=AF.Silu, bias=eb2_sb[:, l:l + 1], scale=1.0)
                        for r in range(G):
                            i = 16 * r + g
                            s = G * g + r
                            nc.tensor.matmul(
                                att_ps[0:NI, :],
                                lhsT=attw_w[:, l, (NI - 1) - i:(2 * NI - 1) - i],
                                rhs=m2slab[:, s * NJ:(s + 1) * NJ],
                                start=(s == 0), stop=(s == NI - 1))

                    # gate: exact sigmoid via tanh (same ACT table as silu)
                    if not do_gate:
                        continue
                    sg = cp1.tile([NI, NJ], F32, tag="sg")
                    nc.scalar.activation(out=sg, in_=att_ps[0:NI, :],
                                         func=AF.Tanh, bias=0.0, scale=0.5)
                    gmask = cp1.tile([NI, NJ], F32, tag="gmask")
                    nc.vector.tensor_scalar(out=gmask, in0=sg, scalar1=0.5,
                                            scalar2=0.5, op0=OP.mult,
                                            op1=OP.add)
                    nc.vector.tensor_tensor(out=gmask, in0=gmask, in1=maskc_sb,
                                            op=OP.mult)
                    gmb = cp1.tile([NI, NJ], BF16, tag="gmb")
                    nc.vector.tensor_copy(gmb, gmask)
                    nc.sync.dma_start(out=gdram[l][:], in_=gmb)
                    # partition-broadcast gate rows via DRAM stride-0 reads,
                    # in slab-slot order s = G*g + r  ->  grid row 16*r + g
                    gb_tiles = [None] * NI
                    for g in range(NGRP):
                        for r in range(G):
                            i = 16 * r + g
                            s = G * g + r
                            gb = gbp.tile([H, NJ], BF16, tag="gb")
                            eng = nc.sync if s % 2 == 0 else nc.gpsimd
                            eng.dma_start(
                                out=gb,
                                in_=bass.AP(tensor=gdram[l], offset=i * NJ,
                                            ap=[[0, H], [1, NJ]]))
                            gb_tiles[s] = gb
                    msumT = cp1.tile([H, NI], F32, tag="msumT")
                    mgs = cp1.tile([H, NJ], BF16, tag="mgs")

                # ---- coord pass (f32r) ----
                if do_coord:
                    phi_ps = ps_acc.tile([H, NJ], F32, tag="acc")

                    def msum_row(s):
                        i = 16 * (s % G) + s // G
                        nc.vector.scalar_tensor_tensor(
                            out=mgs, in0=m2slab[:, s * NJ:(s + 1) * NJ],
                            scalar=1.0, in1=gb_tiles[s],
                            op0=OP.mult, op1=OP.mult,
                            accum_out=msumT[:, i:i + 1])

                    for g in range(NGRP):
                        pre = ps_mlp.tile([H, G * 512], F32, tag="mlp")
                        for r in range(G):
                            RMM(pre[:, r * 512:r * 512 + NJ],
                                lhsT=c1hjT_sb[:, l, :], rhs=h_T,
                                start=True, stop=False)
                        mm_d(pre, combC, g)
                        t1c = work.tile([H, G * NJ], F32, tag="t1c")
                        nc.scalar.activation(
                            out=t1c[:].bitcast(F32R)
                            .rearrange("p (r c) -> p r c", r=G),
                            in_=pre.rearrange("p (r c) -> p r c", r=G)[:, :, 0:NJ],
                            func=AF.Silu, bias=cb1_sb[:, l:l + 1], scale=1.0)
                        z2 = ps_mlp.tile([H, G * 512], F32, tag="mlp")
                        RMM(z2[:, 0:512], lhsT=c2T_sb[:, l, :],
                            rhs=t1c[:, 0:512], start=True, stop=True)
                        RMM(z2[:, 512:1024], lhsT=c2T_sb[:, l, :],
                            rhs=t1c[:, 512:1024], start=True, stop=True)
                        RMM(z2[:, 1024:1152], lhsT=c2T_sb[:, l, :],
                            rhs=t1c[:, 1024:1152], start=True, stop=True)
                        t2c = work.tile([H, G * NJ], F32, tag="t2c")
                        nc.scalar.activation(
                            out=t2c[:].bitcast(F32R),
                            in_=z2[:, 0:G * NJ],
                            func=AF.Silu, bias=cb2_sb[:, l:l + 1], scale=1.0)
                        for r in range(G):
                            i = 16 * r + g
                            s = G * g + r
                            RMM(phi_ps[0:NI, :],
                                lhsT=c3w_sb[:, l,
                                            (NI - 1) - i:(2 * NI - 1) - i],
                                rhs=t2c[:, r * NJ:(r + 1) * NJ],
                                start=(s == 0), stop=(s == NI - 1))
                        if do_gate:
                            for r in range(G):
                                msum_row(G * g + r)

                if do_node:
                    # msum AllGather + node MLP (overlaps late coord groups)
                    nc.gpsimd.dma_start(out=mag_in[l][:], in_=msumT)
                    nc.gpsimd.collective_compute(
                        "AllGather", OP.bypass, replica_groups=rg,
                        ins=[mag_in[l][:]], outs=[mag_out[l][:]])
                    msumF = cp1.tile([H, NJ], F32, tag="msumF")
                    nc.gpsimd.dma_start(
                        out=msumF[:].bitcast(F32R)
                        .rearrange("p (r n) -> p r n", r=NC),
                        in_=mag_out[l].rearrange("(r p) n -> p r n", p=H)
                        .bitcast(F32R))
                    z1 = ps_nd.tile([H, 512], F32, tag="nd")
                    RMM(z1[:, 0:NJ], lhsT=nw1hT_sb[:, l, :], rhs=h_T,
                        start=True, stop=False)
                    RMM(z1[:, 0:NJ], lhsT=nw1mT_sb[:, l, :], rhs=msumF,
                        start=False, stop=True)
                    t1n = cp1.tile([H, NJ], F32, tag="t1n")
                    nc.scalar.activation(out=t1n[:].bitcast(F32R),
                                         in_=z1[:, 0:NJ], func=AF.Silu,
                                         bias=nb1_sb[:, l:l + 1], scale=1.0)
                    z2n = ps_nd.tile([H, 512], F32, tag="nd")
                    RMM(z2n[:, 0:NJ], lhsT=nw2T_sb[:, l, :], rhs=t1n,
                        start=True, stop=True)
                    h_T = cp2.tile([H, NJ], F32, tag="hT")
                    nc.vector.tensor_scalar(out=h_T[:].bitcast(F32R),
                                            in0=z2n[:, 0:NJ],
                                            scalar1=nb2_sb[:, l:l + 1],
                                            scalar2=None, op0=OP.add)
                    if l < L - 2:
                        hT_bf = cp2.tile([H, NJ], BF16, tag="hTb")
                        nc.vector.tensor_copy(hT_bf, h_T)
                    # local copy of this core's own h rows
                    z1m = ps_nd.tile([H, 512], F32, tag="nd")
                    nc.tensor.matmul(z1m[:, 0:NI], lhsT=nw1hT_sb[:, l, :],
                                     rhs=h_my, start=True, stop=False)
                    nc.tensor.matmul(z1m[:, 0:NI], lhsT=nw1mT_sb[:, l, :],
                                     rhs=msumT, start=False, stop=True)
                    t1m = cp1.tile([H, NI], F32, tag="t1m")
                    nc.scalar.activation(out=t1m, in_=z1m[:, 0:NI],
                                         func=AF.Silu,
                                         bias=nb1_sb[:, l:l + 1], scale=1.0)
                    z2m = ps_nd.tile([H, 512], F32, tag="nd")
                    nc.tensor.matmul(z2m[:, 0:NI], lhsT=nw2T_sb[:, l, :],
                                     rhs=t1m, start=True, stop=True)
                    h_my = cp2.tile([H, NI], F32, tag="hmy")
                    nc.vector.tensor_scalar(out=h_my[:].bitcast(F32R),
                                            in0=z2m[:, 0:NI],
                                            scalar1=nb2_sb[:, l:l + 1],
                                            scalar2=None, op0=OP.add)

                if do_coord:
                    # ---- phi stream + x update ----
                    phis = cp1.tile([NI, NJ], F32, tag="phis")
                    nc.vector.tensor_scalar(out=phis, in0=phi_ps[0:NI, :],
                                            scalar1=cb3c_sb[:, l:l + 1],
                                            scalar2=None, op0=OP.add)
                    s = cp1.tile([NI, NJ], F32, tag="s")
                    nc.vector.tensor_tensor(out=s, in0=phis, in1=u, op=OP.mult)
                    xnew = cp2.tile([NI, D], F32, tag="xnew")
                    xms = cp1.tile([NI, NJ], F32, tag="xms")
                    for c in range(D):
                        xcol = cp1.tile([NI, 1], F32, tag=f"xcol{c}")
                        nc.vector.scalar_tensor_tensor(
                            out=xms, in0=diff[c], scalar=1.0, in1=s,
                            op0=OP.mult, op1=OP.mult, accum_out=xcol)
                        nc.vector.tensor_tensor(
                            out=xnew[:, c:c + 1], in0=xcol,
                            in1=x_my[:, c:c + 1], op=OP.add)
                    if not last:
                        nc.gpsimd.dma_start(
                            out=xag_in[l].rearrange("c n -> n c"), in_=xnew)
                    else:
                        nc.gpsimd.dma_start(out=xag_in[l][:], in_=xnew)
                    nc.gpsimd.collective_compute(
                        "AllGather", OP.bypass, replica_groups=rg,
                        ins=[xag_in[l][:]], outs=[xag_out[l][:]])
                    if not last:
                        x_my = xnew
                    else:
                        nc.sync.dma_start(out=o_x[:], in_=xag_out[l][:])

            if KTRUNC:
                nc.sync.dma_start(out=o_x[0:NI, :], in_=x0my_sb)

    nc.finalize()
    return nc


def _prep_inputs(inputs):
    import ml_dtypes
    BF = ml_dtypes.bfloat16
    f = lambda a: np.ascontiguousarray(np.asarray(a), dtype=np.float32)
    b = lambda a: np.ascontiguousarray(np.asarray(a, dtype=np.float32)
                                       .astype(BF))
    x_inp = f(inputs["x_inp"])
    emb_w = f(inputs["emb_w"])
    emb_b = f(inputs["emb_b"])
    coord_w1 = f(inputs["coord_w1"])
    coord_b1 = f(inputs["coord_b1"])
    coord_w2 = f(inputs["coord_w2"])
    coord_b2 = f(inputs["coord_b2"])
    coord_w3 = f(inputs["coord_w3"])
    coord_b3 = f(inputs["coord_b3"])
    edge_w1 = f(inputs["edge_w1"])
    edge_b1 = f(inputs["edge_b1"])
    edge_w2 = f(inputs["edge_w2"])
    edge_b2 = f(inputs["edge_b2"])
    node_w1 = f(inputs["node_w1"])
    node_b1 = f(inputs["node_b1"])
    node_w2 = f(inputs["node_w2"])
    node_b2 = f(inputs["node_b2"])
    att_w = f(inputs["att_w"])

    x0 = x_inp.reshape(N, D)
    eye = np.eye(N, dtype=np.float32)

    def stackT(w, lo, hi):
        return np.ascontiguousarray(
            np.stack([w[l, :, lo:hi].T for l in range(w.shape[0])]))

    def win(w3):
        nl = w3.shape[0]
        out = np.zeros((nl, H, 2 * NI - 1), np.float32)
        out[:, :, NI - 1] = w3[:, 0, :]
        return out

    shared = dict(
        x0rows=np.ascontiguousarray(x0.T.reshape(1, D * N)),
        c1hiT=stackT(coord_w1, 0, H),
        c1hjT=stackT(coord_w1, H, 2 * H),
        cb1=np.ascontiguousarray(coord_b1.T),
        cb2=np.ascontiguousarray(coord_b2.T),
        c2T=np.ascontiguousarray(np.stack([coord_w2[l].T for l in range(L)])),
        c3w=win(coord_w3),
        cb3c=np.ascontiguousarray(
            np.broadcast_to(coord_b3[:, 0][None, :], (NI, L))),
        e1hiT=stackT(edge_w1, 0, H),
        eb1=np.ascontiguousarray(edge_b1.T),
        eb2=np.ascontiguousarray(edge_b2.T),
        nw1hT=stackT(node_w1, 0, H),
        nw1mT=stackT(node_w1, H, 2 * H),
        nb1=np.ascontiguousarray(node_b1.T),
        nw2T=np.ascontiguousarray(np.stack([node_w2[l].T for l in range(L - 1)])),
        nb2=np.ascontiguousarray(node_b2.T),
        cdrep=np.ascontiguousarray(np.tile(coord_w1[:, :, 2 * H], (1, 16))),
        edrep=np.ascontiguousarray(np.tile(edge_w1[:, :, 2 * H], (1, 16))),
        ones6k=np.ones((1, 16 * NJ), np.float32),
        e1hjT_b=b(stackT(edge_w1, H, 2 * H)),
        e2T_b=b(np.stack([edge_w2[l].T for l in range(L - 1)])),
        attw_b=b(win(att_w)),
    )
    in_maps = []
    for c in range(NC):
        m = dict(shared)
        m["embw"] = np.ascontiguousarray(
            emb_w[c * EMB_ROWS:(c + 1) * EMB_ROWS, :])
        m["embbT"] = np.ascontiguousarray(
            emb_b[c * EMB_ROWS:(c + 1) * EMB_ROWS].reshape(NI, H).T)
        m["x0my"] = np.ascontiguousarray(x0[c * NI:(c + 1) * NI, :])
        m["maskc"] = np.ascontiguousarray(1.0 - eye[c * NI:(c + 1) * NI, :])
        m["eyec"] = np.ascontiguousarray(eye[c * NI:(c + 1) * NI, :])
        in_maps.append(m)
    return in_maps


def _run(inputs, trace=False, **kw):
    from concourse.bass_utils import run_bass_kernel_spmd
    if "nc" not in _cache:
        _cache["nc"] = _build_nc()
    in_maps = _prep_inputs(inputs)
    return run_bass_kernel_spmd(_cache["nc"], in_maps, list(range(NC)),
                                trace=trace, **kw)


def kernel(**inputs) -> np.ndarray:
    res = _run(inputs)
    return np.asarray(res.results[0]["o_x"], dtype=np.float32).reshape(N * D)


# revision 16
# speedup vs baseline: 1.5851x; 1.0640x over previous
"""EGNN (N=384, D=3, H=128, L=4) Bass kernel for 8 TRN2 NeuronCores.

Sharding: rows of the N x N edge grid split across 8 cores (48 rows each).
Per layer each core computes its row-block of the coord/edge MLPs and
row-sums, AllGathers the per-node x updates and msum rows; the node MLP is
computed redundantly per core. The embedding row-sum of emb_w (the dominant
HBM traffic) is sharded 1/8 per core.

Key perf structure (PE observed pinned at 1.2 GHz = 1 cyc/row for both
f32r and bf16, so coord-pass f32r precision is free):
- G=3 row groups; group g covers rows {g, 16+g, 32+g} so the three rank-2
  [w1d; A_i] matmuls land in PE row-quadrants 0/32/64 and run concurrently.
- Edge pass bf16 (halves SBUF for the m2 slab), coord pass f32r.
- Gate sigmoid via tanh (same ACT table as silu -> zero table swaps);
  sqrt via bit-trick + Newton using reciprocal_approx_fast on the DVE.
- Gated msum: gate rows partition-broadcast via DRAM stride-0 DMA, then a
  fused scalar_tensor_tensor accumulation per row (no PE, no PSUM).
- Embedding: 8 large DMAs issued up-front on 3 queues; layer-0 coordinate
  prep hoisted before the reduces to fill the DVE while DMAs stream.
"""
import os
import numpy as np

KTRUNC = int(os.environ.get("KTRUNC", "0"))

N, D, H, L = 384, 3, 128, 4
NC = 8
NI = N // NC          # 48 rows per core
NJ = N                # 384 cols
G = 3                 # rows per group (one per PE row-quadrant)
NGRP = NI // G        # 16 groups
EB = 6                # nodes per embedding DMA
NEMB = NI // EB       # 8 embedding tiles
EMB_ROWS = N * H // NC

QUAKE = 0x1FBD1DF5

_cache = {}


def _build_nc():
    import concourse.bass as bass
    import concourse.bacc as bacc
    import concourse.tile as tile
    from concourse import mybir

    F32 = mybir.dt.float32
    F32R = mybir.dt.float32r
    BF16 = mybir.dt.bfloat16
    I32 = mybir.dt.int32
    AF = mybir.ActivationFunctionType
    OP = mybir.AluOpType

    nc = bacc.Bacc(None, target_bir_lowering=False)

    def RMM(out, lhsT, rhs, **kw):
        nc.tensor.matmul(out, lhsT=lhsT.bitcast(F32R), rhs=rhs.bitcast(F32R), **kw)

    def P(name, shape, dt=F32):
        return nc.declare_dram_parameter(name, list(shape), dt, isOutput=False)

    # per-core inputs
    embw = P("embw", (EMB_ROWS, NJ))
    embbT = P("embbT", (H, NI))
    x0my = P("x0my", (NI, D))
    maskc = P("maskc", (NI, NJ))
    eyec = P("eyec", (NI, NJ))
    # shared inputs (f32)
    x0rows = P("x0rows", (1, D * NJ))
    c1hiT = P("c1hiT", (L, H, H))
    c1hjT = P("c1hjT", (L, H, H))
    cb1 = P("cb1", (H, L))
    cb2 = P("cb2", (H, L))
    c2T = P("c2T", (L, H, H))
    c3w = P("c3w", (L, H, 2 * NI - 1))
    cb3c = P("cb3c", (NI, L))
    e1hiT = P("e1hiT", (L - 1, H, H))
    eb1 = P("eb1", (H, L - 1))
    eb2 = P("eb2", (H, L - 1))
    nw1hT = P("nw1hT", (L - 1, H, H))
    nw1mT = P("nw1mT", (L - 1, H, H))
    nb1 = P("nb1", (H, L - 1))
    nw2T = P("nw2T", (L - 1, H, H))
    nb2 = P("nb2", (H, L - 1))
    cdrep = P("cdrep", (L, 16 * H))
    edrep = P("edrep", (L - 1, 16 * H))
    ones6k = P("ones6k", (1, 16 * NJ))
    # shared inputs (bf16, edge pass)
    e1hjT_b = P("e1hjT_b", (L - 1, H, H), BF16)
    e2T_b = P("e2T_b", (L - 1, H, H), BF16)
    attw_b = P("attw_b", (L - 1, H, 2 * NI - 1), BF16)

    o_x = nc.declare_dram_parameter("o_x", [N, D], F32, isOutput=True)

    # DRAM internals
    gdram = [nc.dram_tensor(f"gdram{l}", [NI * NJ], BF16) for l in range(L - 1)]
    hag_in = nc.dram_tensor("hag_in", [H, NI], F32)
    hag_out = nc.dram_tensor("hag_out", [NC * H, NI], F32, addr_space="Shared")
    xag_in = [nc.dram_tensor(f"xag_in{l}", [D, NI], F32) for l in range(L - 1)]
    xag_in.append(nc.dram_tensor("xag_in3", [NI, D], F32))
    xag_out = [nc.dram_tensor(f"xag_out{l}", [NC, D, NI], F32, addr_space="Shared")
               for l in range(L - 1)]
    xag_out.append(nc.dram_tensor("xag_out3", [N, D], F32, addr_space="Shared"))
    bar_in = nc.dram_tensor("bar_in", [1, 1], F32)
    bar_out = nc.dram_tensor("bar_out", [NC, 1], F32, addr_space="Shared")
    mag_in = [nc.dram_tensor(f"mag_in{l}", [H, NI], F32) for l in range(L - 1)]
    mag_out = [nc.dram_tensor(f"mag_out{l}", [NC * H, NI], F32, addr_space="Shared")
               for l in range(L - 1)]
    rg = [list(range(NC))]

    with tile.TileContext(nc) as tc:
        with (
            tc.tile_pool(name="consts", bufs=1) as consts,
            tc.tile_pool(name="embp", bufs=3) as embp,
            tc.tile_pool(name="cp1", bufs=1) as cp1,
            tc.tile_pool(name="cp2", bufs=2) as cp2,
            tc.tile_pool(name="work", bufs=2) as work,
            tc.tile_pool(name="slab", bufs=1) as slabp,
            tc.tile_pool(name="gbp", bufs=6) as gbp,
            tc.tile_pool(name="ps_mlp", bufs=2, space="PSUM") as ps_mlp,
            tc.tile_pool(name="ps_acc", bufs=1, space="PSUM") as ps_acc,
            tc.tile_pool(name="ps_nd", bufs=1, space="PSUM") as ps_nd,
        ):
            # ---- phase 0 ----
            # A tiny leading AllGather absorbs one-time cross-core launch
            # skew while the embedding DMAs stream on sync/scalar queues.
            nc.gpsimd.collective_compute(
                "AllGather", OP.bypass, replica_groups=rg,
                ins=[bar_in[:]], outs=[bar_out[:]])
            emb_tiles = []
            qrot = [nc.sync, nc.scalar]
            for t in range(NEMB):
                et = embp.tile([H, EB, NJ], F32, tag="embt")
                qrot[t % 2].dma_start(
                    out=et,
                    in_=embw[t * EB * H:(t + 1) * EB * H, :]
                    .rearrange("(a p) j -> p a j", p=H))
                emb_tiles.append(et)

            # ---- constants (scalar queue; ACT is idle during emb) ----
            def load(pname, ap_in, shape, dt=F32, rnd=False):
                t = consts.tile(list(shape), dt, tag=pname)
                o = t[:].bitcast(F32R) if rnd else t
                nc.scalar.dma_start(
                    out=o, in_=ap_in.bitcast(F32R) if rnd else ap_in)
                return t

            x0my_sb = load("x0my", x0my[:], (NI, D))
            embbT_sb = load("embbT", embbT[:], (H, NI))
            maskc_sb = load("maskc", maskc[:], (NI, NJ))
            eyec_sb = load("eyec", eyec[:], (NI, NJ))
            c1hiT_sb = load("c1hiT", c1hiT.rearrange("l p x -> p l x"), (H, L, H),
                            rnd=True)
            c1hjT_sb = load("c1hjT", c1hjT.rearrange("l p x -> p l x"), (H, L, H),
                            rnd=True)
            c2T_sb = load("c2T", c2T.rearrange("l p x -> p l x"), (H, L, H),
                          rnd=True)
            c3w_sb = load("c3w", c3w.rearrange("l p x -> p l x"),
                          (H, L, 2 * NI - 1), rnd=True)
            cb1_sb = load("cb1", cb1[:], (H, L))
            cb2_sb = load("cb2", cb2[:], (H, L))
            cb3c_sb = load("cb3c", cb3c[:], (NI, L))
            e1hiT_sb = load("e1hiT", e1hiT.rearrange("l p x -> p l x"),
                            (H, L - 1, H), rnd=True)
            eb1_sb = load("eb1", eb1[:], (H, L - 1))
            eb2_sb = load("eb2", eb2[:], (H, L - 1))
            nw1hT_sb = load("nw1hT", nw1hT.rearrange("l p x -> p l x"),
                            (H, L - 1, H), rnd=True)
            nw1mT_sb = load("nw1mT", nw1mT.rearrange("l p x -> p l x"),
                            (H, L - 1, H), rnd=True)
            nb1_sb = load("nb1", nb1[:], (H, L - 1))
            nw2T_sb = load("nw2T", nw2T.rearrange("l p x -> p l x"),
                           (H, L - 1, H), rnd=True)
            nb2_sb = load("nb2", nb2[:], (H, L - 1))
            cdrep_sb = load("cdrep", cdrep[:], (L, 16 * H), rnd=True)
            edrep_sb = load("edrep", edrep[:], (L - 1, 16 * H), rnd=True)
            e1hjT_w = load("e1hjT_b", e1hjT_b.rearrange("l p x -> p l x"),
                           (H, L - 1, H), BF16)
            e2T_w = load("e2T_b", e2T_b.rearrange("l p x -> p l x"),
                         (H, L - 1, H), BF16)
            attw_w = load("attw_b", attw_b.rearrange("l p x -> p l x"),
                          (H, L - 1, 2 * NI - 1), BF16)

            # d2/ones interleaved stripes (f32r): rows 32k = d2 rows
            # 16k..16k+15 flattened, rows 32k+1 = ones.
            d2ones = consts.tile([66, 16 * NJ], F32, tag="d2ones")
            combC = consts.tile([66, 16 * H], F32, tag="combC")
            combE = consts.tile([66, 16 * H], F32, tag="combE")
            for k in range(3):
                nc.scalar.dma_start(
                    out=d2ones[32 * k + 1:32 * k + 2, :].bitcast(F32R),
                    in_=ones6k[:].bitcast(F32R))

            def coord_prep(l, x_my):
                """xb loads + diff/d2/u chain + d2 stripes for layer l."""
                diff = []
                for c in range(D):
                    xb = cp1.tile([NI, NJ], F32, tag=f"xb{c}")
                    if l == 0:
                        bsrc = bass.AP(tensor=x0rows, offset=c * NJ,
                                       ap=[[0, NI], [1, NJ]])
                    else:
                        bsrc = bass.AP(tensor=xag_out[l - 1], offset=c * NI,
                                       ap=[[0, NI], [D * NI, NC], [1, NI]])
                    nc.sync.dma_start(out=xb, in_=bsrc)
                    dc = cp2.tile([NI, NJ], F32, tag=f"diff{c}")
                    nc.vector.tensor_scalar(
                        out=dc, in0=xb, scalar1=x_my[:, c:c + 1], scalar2=None,
                        op0=OP.subtract)
                    diff.append(dc)
                d2 = cp1.tile([NI, NJ], F32, tag="d2")
                tmp = cp1.tile([NI, NJ], F32, tag="ctmp")
                nc.vector.tensor_tensor(out=d2, in0=diff[0], in1=diff[0],
                                        op=OP.mult)
                nc.vector.tensor_tensor(out=tmp, in0=diff[1], in1=diff[1],
                                        op=OP.mult)
                nc.vector.tensor_tensor(out=d2, in0=d2, in1=tmp, op=OP.add)
                nc.vector.tensor_tensor(out=tmp, in0=diff[2], in1=diff[2],
                                        op=OP.mult)
                nc.vector.tensor_tensor(out=d2, in0=d2, in1=tmp, op=OP.add)
                for k in range(3):
                    nc.sync.dma_start(
                        out=d2ones[32 * k:32 * k + 1, :].bitcast(F32R),
                        in_=d2[16 * k:16 * (k + 1), :].bitcast(F32R))
                d2s = cp1.tile([NI, NJ], F32, tag="d2s")
                nc.vector.tensor_tensor(out=d2s, in0=d2, in1=eyec_sb, op=OP.add)
                # sqrt via bit-trick seed + 1 Newton step (approx recip)
                sq = cp1.tile([NI, NJ], F32, tag="sq")
                with nc.allow_low_precision(reason="bit-trick sqrt"):
                    nc.vector.tensor_scalar(
                        out=sq.bitcast(I32), in0=d2s.bitcast(I32),
                        scalar1=1, scalar2=None, op0=OP.logical_shift_right)
                    nc.vector.tensor_scalar(
                        out=sq.bitcast(I32), in0=sq.bitcast(I32),
                        scalar1=QUAKE, scalar2=None, op0=OP.add)
                nc.vector.reciprocal_approx_fast(out=tmp, in_=sq)
                nc.vector.tensor_tensor(out=tmp, in0=d2s, in1=tmp, op=OP.mult)
                nc.vector.tensor_tensor(out=sq, in0=sq, in1=tmp, op=OP.add)
                nc.vector.tensor_scalar(out=sq, in0=sq, scalar1=0.5,
                                        scalar2=None, op0=OP.mult)
                nc.vector.tensor_scalar(out=sq, in0=sq, scalar1=1.0,
                                        scalar2=None, op0=OP.add)
                u = cp2.tile([NI, NJ], F32, tag="u")
                nc.vector.reciprocal_approx_fast(out=u, in_=sq)
                nc.vector.tensor_tensor(out=u, in0=u, in1=maskc_sb, op=OP.mult)
                return diff, u

            prep0 = coord_prep(0, x0my_sb)

            # ---- embedding reduce + h AllGather ----
            hT0 = cp1.tile([H, NI], F32, tag="hT0")
            for t in range(NEMB):
                with nc.allow_low_precision(reason="f32r storage, f32 accum"):
                    nc.vector.tensor_reduce(
                        out=hT0[:, t * EB:(t + 1) * EB].bitcast(F32R),
                        in_=emb_tiles[t],
                        axis=mybir.AxisListType.X, op=OP.add)
            nc.vector.tensor_tensor(out=hT0[:].bitcast(F32R), in0=hT0,
                                    in1=embbT_sb, op=OP.add)
            nc.gpsimd.dma_start(out=hag_in[:], in_=hT0)
            nc.gpsimd.collective_compute(
                "AllGather", OP.bypass, replica_groups=rg,
                ins=[hag_in[:]], outs=[hag_out[:]])
            h_T = cp2.tile([H, NJ], F32, tag="hT")
            nc.scalar.dma_start(
                out=h_T[:].bitcast(F32R).rearrange("p (r n) -> p r n", r=NC),
                in_=hag_out.rearrange("(r p) n -> p r n", p=H).bitcast(F32R))
            h_my = hT0
            hT_bf = cp2.tile([H, NJ], BF16, tag="hTb")
            nc.vector.tensor_copy(hT_bf, h_T)

            x_my = x0my_sb

            for l in range(L):
                if KTRUNC == 1 or (KTRUNC and l > 0):
                    break
                last = l == L - 1
                do_gate = (not last) and KTRUNC in (0, 3, 4)
                do_coord = KTRUNC in (0, 3, 4)
                do_node = (not last) and KTRUNC in (0, 4)

                diff, u = prep0 if l == 0 else coord_prep(l, x_my)

                # A-terms: At[i, :] = (W1hi @ h_i), striped into comb pairs
                def make_comb(hiT, wdrep_row, comb, atag):
                    At_ps = ps_nd.tile([NI, H], F32, tag="nd")
                    RMM(At_ps, lhsT=h_my, rhs=hiT, start=True, stop=True)
                    At = cp1.tile([NI, H], F32, tag=atag)
                    nc.vector.tensor_copy(At[:].bitcast(F32R), At_ps)
                    for k in range(3):
                        nc.sync.dma_start(
                            out=comb[32 * k:32 * k + 1, :].bitcast(F32R),
                            in_=wdrep_row.bitcast(F32R))
                        nc.sync.dma_start(
                            out=comb[32 * k + 1:32 * k + 2, :].bitcast(F32R),
                            in_=At[16 * k:16 * (k + 1), :].bitcast(F32R))

                make_comb(c1hiT_sb[:, l, :], cdrep_sb[l:l + 1, :], combC, "Atc")
                if not last:
                    make_comb(e1hiT_sb[:, l, :], edrep_sb[l:l + 1, :], combE,
                              "Ate")

                # group g covers rows {g, 16+g, 32+g}: the three rank-2
                # matmuls hit PE row-quadrants 0/32/64 and run concurrently
                def mm_d(pre, comb, g):
                    for r in range(G):
                        nc.tensor.matmul(
                            pre[:, r * 512:r * 512 + NJ],
                            lhsT=comb[32 * r:32 * r + 2,
                                      g * H:(g + 1) * H].bitcast(F32R),
                            rhs=d2ones[32 * r:32 * r + 2,
                                       g * NJ:(g + 1) * NJ].bitcast(F32R),
                            start=False, stop=True)

                # ---- edge pass (bf16) ----
                if not last:
                    att_ps = ps_acc.tile([H, NJ], F32, tag="acc")
                    m2slab = slabp.tile([H, NI * NJ], BF16, tag="m2")

                    def edge_tail(item):
                        t1, g = item
                        z2 = ps_mlp.tile([H, G * 512], F32, tag="mlp")
                        nc.tensor.matmul(z2[:, 0:512], lhsT=e2T_w[:, l, :],
                                         rhs=t1[:, 0:512], start=True,
                                         stop=True)
                        nc.tensor.matmul(z2[:, 512:1024], lhsT=e2T_w[:, l, :],
                                         rhs=t1[:, 512:1024], start=True,
                                         stop=True)
                        nc.tensor.matmul(z2[:, 1024:1152], lhsT=e2T_w[:, l, :],
                                         rhs=t1[:, 1024:1152], start=True,
                                         stop=True)
                        nc.scalar.activation(
                            out=m2slab[:, g * G * NJ:(g + 1) * G * NJ],
                            in_=z2[:, 0:G * NJ],
                            func=AF.Silu, bias=eb2_sb[:, l:l + 1], scale=1.0)
                        for r in range(G):
                            i = 16 * r + g
                            s = G * g + r
                            nc.tensor.matmul(
                                att_ps[0:NI, :],
                                lhsT=attw_w[:, l, (NI - 1) - i:(2 * NI - 1) - i],
                                rhs=m2slab[:, s * NJ:(s + 1) * NJ],
                                start=(s == 0), stop=(s == NI - 1))

                    epend = None
                    for g in range(NGRP):
                        pre = ps_mlp.tile([H, G * 512], F32, tag="mlp")
                        for r in range(G):
                            nc.tensor.matmul(
                                pre[:, r * 512:r * 512 + NJ],
                                lhsT=e1hjT_w[:, l, :], rhs=hT_bf,
                                start=True, stop=False)
                        mm_d(pre, combE, g)
                        t1 = work.tile([H, G * NJ], BF16, tag="t1")
                        nc.scalar.activation(
                            out=t1.rearrange("p (r c) -> p r c", r=G),
                            in_=pre.rearrange("p (r c) -> p r c", r=G)[:, :, 0:NJ],
                            func=AF.Silu, bias=eb1_sb[:, l:l + 1], scale=1.0)
                        if epend is not None:
                            edge_tail(epend)
                        epend = (t1, g)
                    edge_tail(epend)

                    # gate: exact sigmoid via tanh (same ACT table as silu)
                    if not do_gate:
                        continue
                    sg = cp1.tile([NI, NJ], F32, tag="sg")
                    nc.scalar.activation(out=sg, in_=att_ps[0:NI, :],
                                         func=AF.Tanh, bias=0.0, scale=0.5)
                    gmask = cp1.tile([NI, NJ], F32, tag="gmask")
                    nc.vector.tensor_scalar(out=gmask, in0=sg, scalar1=0.5,
                                            scalar2=0.5, op0=OP.mult,
                                            op1=OP.add)
                    nc.vector.tensor_tensor(out=gmask, in0=gmask, in1=maskc_sb,
                                            op=OP.mult)
                    gmb = cp1.tile([NI, NJ], BF16, tag="gmb")
                    nc.vector.tensor_copy(gmb, gmask)
                    nc.sync.dma_start(out=gdram[l][:], in_=gmb)
                    # partition-broadcast gate rows via DRAM stride-0 reads,
                    # in slab-slot order s = G*g + r  ->  grid row 16*r + g
                    gb_tiles = [None] * NI
                    for g in range(NGRP):
                        for r in range(G):
                            i = 16 * r + g
                            s = G * g + r
                            gb = gbp.tile([H, NJ], BF16, tag="gb")
                            eng = nc.sync if s % 2 == 0 else nc.gpsimd
                            eng.dma_start(
                                out=gb,
                                in_=bass.AP(tensor=gdram[l], offset=i * NJ,
                                            ap=[[0, H], [1, NJ]]))
                            gb_tiles[s] = gb
                    msumT = cp1.tile([H, NI], F32, tag="msumT")
                    mgs = cp1.tile([H, NJ], BF16, tag="mgs")

                # ---- coord pass (f32r) ----
                if do_coord:
                    phi_ps = ps_acc.tile([H, NJ], F32, tag="acc")

                    def msum_row(s):
                        i = 16 * (s % G) + s // G
                        nc.vector.scalar_tensor_tensor(
                            out=mgs, in0=m2slab[:, s * NJ:(s + 1) * NJ],
                            scalar=1.0, in1=gb_tiles[s],
                            op0=OP.mult, op1=OP.mult,
                            accum_out=msumT[:, i:i + 1])

                    def coord_tail(item):
                        t1c, g = item
                        z2 = ps_mlp.tile([H, G * 512], F32, tag="mlp")
                        RMM(z2[:, 0:512], lhsT=c2T_sb[:, l, :],
                            rhs=t1c[:, 0:512], start=True, stop=True)
                        RMM(z2[:, 512:1024], lhsT=c2T_sb[:, l, :],
                            rhs=t1c[:, 512:1024], start=True, stop=True)
                        RMM(z2[:, 1024:1152], lhsT=c2T_sb[:, l, :],
                            rhs=t1c[:, 1024:1152], start=True, stop=True)
                        t2c = work.tile([H, G * NJ], F32, tag="t2c")
                        nc.scalar.activation(
                            out=t2c[:].bitcast(F32R),
                            in_=z2[:, 0:G * NJ],
                            func=AF.Silu, bias=cb2_sb[:, l:l + 1], scale=1.0)
                        for r in range(G):
                            i = 16 * r + g
                            s = G * g + r
                            RMM(phi_ps[0:NI, :],
                                lhsT=c3w_sb[:, l,
                                            (NI - 1) - i:(2 * NI - 1) - i],
                                rhs=t2c[:, r * NJ:(r + 1) * NJ],
                                start=(s == 0), stop=(s == NI - 1))
                        if do_gate:
                            for r in range(G):
                                msum_row(G * g + r)

                    cpend = None
                    for g in range(NGRP):
                        pre = ps_mlp.tile([H, G * 512], F32, tag="mlp")
                        for r in range(G):
                            RMM(pre[:, r * 512:r * 512 + NJ],
                                lhsT=c1hjT_sb[:, l, :], rhs=h_T,
                                start=True, stop=False)
                        mm_d(pre, combC, g)
                        t1c = work.tile([H, G * NJ], F32, tag="t1c")
                        nc.scalar.activation(
                            out=t1c[:].bitcast(F32R)
                            .rearrange("p (r c) -> p r c", r=G),
                            in_=pre.rearrange("p (r c) -> p r c", r=G)[:, :, 0:NJ],
                            func=AF.Silu, bias=cb1_sb[:, l:l + 1], scale=1.0)
                        if cpend is not None:
                            coord_tail(cpend)
                        cpend = (t1c, g)
                    coord_tail(cpend)

                if do_node:
                    # msum AllGather + node MLP (overlaps late coord groups)
                    nc.gpsimd.dma_start(out=mag_in[l][:], in_=msumT)
                    nc.gpsimd.collective_compute(
                        "AllGather", OP.bypass, replica_groups=rg,
                        ins=[mag_in[l][:]], outs=[mag_out[l][:]])
                    msumF = cp1.tile([H, NJ], F32, tag="msumF")
                    nc.gpsimd.dma_start(
                        out=msumF[:].bitcast(F32R)
                        .rearrange("p (r n) -> p r n", r=NC),
                        in_=mag_out[l].rearrange("(r p) n -> p r n", p=H)
                        .bitcast(F32R))
                    z1 = ps_nd.tile([H, 512], F32, tag="nd")
                    RMM(z1[:, 0:NJ], lhsT=nw1hT_sb[:, l, :], rhs=h_T,
                        start=True, stop=False)
                    RMM(z1[:, 0:NJ], lhsT=nw1mT_sb[:, l, :], rhs=msumF,
                        start=False, stop=True)
                    t1n = cp1.tile([H, NJ], F32, tag="t1n")
                    nc.scalar.activation(out=t1n[:].bitcast(F32R),
                                         in_=z1[:, 0:NJ], func=AF.Silu,
                                         bias=nb1_sb[:, l:l + 1], scale=1.0)
                    z2n = ps_nd.tile([H, 512], F32, tag="nd")
                    RMM(z2n[:, 0:NJ], lhsT=nw2T_sb[:, l, :], rhs=t1n,
                        start=True, stop=True)
                    h_T = cp2.tile([H, NJ], F32, tag="hT")
                    nc.vector.tensor_scalar(out=h_T[:].bitcast(F32R),
                                            in0=z2n[:, 0:NJ],
                                            scalar1=nb2_sb[:, l:l + 1],
                                            scalar2=None, op0=OP.add)
                    if l < L - 2:
                        hT_bf = cp2.tile([H, NJ], BF16, tag="hTb")
                        nc.vector.tensor_copy(hT_bf, h_T)
                    # local copy of this core's own h rows
                    z1m = ps_nd.tile([H, 512], F32, tag="nd")
                    nc.tensor.matmul(z1m[:, 0:NI], lhsT=nw1hT_sb[:, l, :],
                                     rhs=h_my, start=True, stop=False)
                    nc.tensor.matmul(z1m[:, 0:NI], lhsT=nw1mT_sb[:, l, :],
                                     rhs=msumT, start=False, stop=True)
                    t1m = cp1.tile([H, NI], F32, tag="t1m")
                    nc.scalar.activation(out=t1m, in_=z1m[:, 0:NI],
                                         func=AF.Silu,
                                         bias=nb1_sb[:, l:l + 1], scale=1.0)
                    z2m = ps_nd.tile([H, 512], F32, tag="nd")
                    nc.tensor.matmul(z2m[:, 0:NI], lhsT=nw2T_sb[:, l, :],
                                     rhs=t1m, start=True, stop=True)
                    h_my = cp2.tile([H, NI], F32, tag="hmy")
                    nc.vector.tensor_scalar(out=h_my[:].bitcast(F32R),
                                            in0=z2m[:, 0:NI],
                                            scalar1=nb2_sb[:, l:l + 1],
                                            scalar2=None, op0=OP.add)

                if do_coord:
                    # ---- phi stream + x update ----
                    phis = cp1.tile([NI, NJ], F32, tag="phis")
                    nc.vector.tensor_scalar(out=phis, in0=phi_ps[0:NI, :],
                                            scalar1=cb3c_sb[:, l:l + 1],
                                            scalar2=None, op0=OP.add)
                    s = cp1.tile([NI, NJ], F32, tag="s")
                    nc.vector.tensor_tensor(out=s, in0=phis, in1=u, op=OP.mult)
                    xnew = cp2.tile([NI, D], F32, tag="xnew")
                    xms = cp1.tile([NI, NJ], F32, tag="xms")
                    for c in range(D):
                        xcol = cp1.tile([NI, 1], F32, tag=f"xcol{c}")
                        nc.vector.scalar_tensor_tensor(
                            out=xms, in0=diff[c], scalar=1.0, in1=s,
                            op0=OP.mult, op1=OP.mult, accum_out=xcol)
                        nc.vector.tensor_tensor(
                            out=xnew[:, c:c + 1], in0=xcol,
                            in1=x_my[:, c:c + 1], op=OP.add)
                    if not last:
                        nc.gpsimd.dma_start(
                            out=xag_in[l].rearrange("c n -> n c"), in_=xnew)
                    else:
                        nc.gpsimd.dma_start(out=xag_in[l][:], in_=xnew)
                    nc.gpsimd.collective_compute(
                        "AllGather", OP.bypass, replica_groups=rg,
                        ins=[xag_in[l][:]], outs=[xag_out[l][:]])
                    if not last:
                        x_my = xnew
                    else:
                        nc.sync.dma_start(out=o_x[:], in_=xag_out[l][:])

            if KTRUNC:
                nc.sync.dma_start(out=o_x[0:NI, :], in_=x0my_sb)

    nc.finalize()
    return nc


def _prep_inputs(inputs):
    import ml_dtypes
    BF = ml_dtypes.bfloat16
    f = lambda a: np.ascontiguousarray(np.asarray(a), dtype=np.float32)
    b = lambda a: np.ascontiguousarray(np.asarray(a, dtype=np.float32)
                                       .astype(BF))
    x_inp = f(inputs["x_inp"])
    emb_w = f(inputs["emb_w"])
    emb_b = f(inputs["emb_b"])
    coord_w1 = f(inputs["coord_w1"])
    coord_b1 = f(inputs["coord_b1"])
    coord_w2 = f(inputs["coord_w2"])
    coord_b2 = f(inputs["coord_b2"])
    coord_w3 = f(inputs["coord_w3"])
    coord_b3 = f(inputs["coord_b3"])
    edge_w1 = f(inputs["edge_w1"])
    edge_b1 = f(inputs["edge_b1"])
    edge_w2 = f(inputs["edge_w2"])
    edge_b2 = f(inputs["edge_b2"])
    node_w1 = f(inputs["node_w1"])
    node_b1 = f(inputs["node_b1"])
    node_w2 = f(inputs["node_w2"])
    node_b2 = f(inputs["node_b2"])
    att_w = f(inputs["att_w"])

    x0 = x_inp.reshape(N, D)
    eye = np.eye(N, dtype=np.float32)

    def stackT(w, lo, hi):
        return np.ascontiguousarray(
            np.stack([w[l, :, lo:hi].T for l in range(w.shape[0])]))

    def win(w3):
        nl = w3.shape[0]
        out = np.zeros((nl, H, 2 * NI - 1), np.float32)
        out[:, :, NI - 1] = w3[:, 0, :]
        return out

    shared = dict(
        x0rows=np.ascontiguousarray(x0.T.reshape(1, D * N)),
        c1hiT=stackT(coord_w1, 0, H),
        c1hjT=stackT(coord_w1, H, 2 * H),
        cb1=np.ascontiguousarray(coord_b1.T),
        cb2=np.ascontiguousarray(coord_b2.T),
        c2T=np.ascontiguousarray(np.stack([coord_w2[l].T for l in range(L)])),
        c3w=win(coord_w3),
        cb3c=np.ascontiguousarray(
            np.broadcast_to(coord_b3[:, 0][None, :], (NI, L))),
        e1hiT=stackT(edge_w1, 0, H),
        eb1=np.ascontiguousarray(edge_b1.T),
        eb2=np.ascontiguousarray(edge_b2.T),
        nw1hT=stackT(node_w1, 0, H),
        nw1mT=stackT(node_w1, H, 2 * H),
        nb1=np.ascontiguousarray(node_b1.T),
        nw2T=np.ascontiguousarray(np.stack([node_w2[l].T for l in range(L - 1)])),
        nb2=np.ascontiguousarray(node_b2.T),
        cdrep=np.ascontiguousarray(np.tile(coord_w1[:, :, 2 * H], (1, 16))),
        edrep=np.ascontiguousarray(np.tile(edge_w1[:, :, 2 * H], (1, 16))),
        ones6k=np.ones((1, 16 * NJ), np.float32),
        e1hjT_b=b(stackT(edge_w1, H, 2 * H)),
        e2T_b=b(np.stack([edge_w2[l].T for l in range(L - 1)])),
        attw_b=b(win(att_w)),
    )
    in_maps = []
    for c in range(NC):
        m = dict(shared)
        m["embw"] = np.ascontiguousarray(
            emb_w[c * EMB_ROWS:(c + 1) * EMB_ROWS, :])
        m["embbT"] = np.ascontiguousarray(
            emb_b[c * EMB_ROWS:(c + 1) * EMB_ROWS].reshape(NI, H).T)
        m["x0my"] = np.ascontiguousarray(x0[c * NI:(c + 1) * NI, :])
        m["maskc"] = np.ascontiguousarray(1.0 - eye[c * NI:(c + 1) * NI, :])
        m["eyec"] = np.ascontiguousarray(eye[c * NI:(c + 1) * NI, :])
        in_maps.append(m)
    return in_maps


def _run(inputs, trace=False, **kw):
    from concourse.bass_utils import run_bass_kernel_spmd
    if "nc" not in _cache:
        _cache["nc"] = _build_nc()
    in_maps = _prep_inputs(inputs)
    return run_bass_kernel_spmd(_cache["nc"], in_maps, list(range(NC)),
                                trace=trace, **kw)


def kernel(**inputs) -> np.ndarray:
    res = _run(inputs)
    return np.asarray(res.results[0]["o_x"], dtype=np.float32).reshape(N * D)


# revision 18
# speedup vs baseline: 1.6098x; 1.0156x over previous
"""EGNN (N=384, D=3, H=128, L=4) Bass kernel for 8 TRN2 NeuronCores.

Sharding: rows of the N x N edge grid split across 8 cores (48 rows each).
Per layer each core computes its row-block of the coord/edge MLPs and
row-sums, AllGathers the per-node x updates and msum rows; the node MLP is
computed redundantly per core. The embedding row-sum of emb_w (the dominant
HBM traffic) is sharded 1/8 per core.

Key perf structure (PE observed pinned at 1.2 GHz = 1 cyc/row for both
f32r and bf16, so coord-pass f32r precision is free):
- G=3 row groups; group g covers rows {g, 16+g, 32+g} so the three rank-2
  [w1d; A_i] matmuls land in PE row-quadrants 0/32/64 and run concurrently.
- Edge pass bf16 (halves SBUF for the m2 slab), coord pass f32r.
- Gate sigmoid via tanh (same ACT table as silu -> zero table swaps);
  sqrt via bit-trick + Newton using reciprocal_approx_fast on the DVE.
- Gated msum: gate rows partition-broadcast via DRAM stride-0 DMA, then a
  fused scalar_tensor_tensor accumulation per row (no PE, no PSUM).
- Embedding: 8 large DMAs issued up-front on 3 queues; layer-0 coordinate
  prep hoisted before the reduces to fill the DVE while DMAs stream.
"""
import os
import numpy as np

KTRUNC = int(os.environ.get("KTRUNC", "0"))

N, D, H, L = 384, 3, 128, 4
NC = 8
NI = N // NC          # 48 rows per core
NJ = N                # 384 cols
G = 3                 # rows per group (one per PE row-quadrant)
NGRP = NI // G        # 16 groups
EB = 6                # nodes per embedding DMA
NEMB = NI // EB       # 8 embedding tiles
EMB_ROWS = N * H // NC

QUAKE = 0x1FBD1DF5

_cache = {}


def _build_nc():
    import concourse.bass as bass
    import concourse.bacc as bacc
    import concourse.tile as tile
    from concourse import mybir

    F32 = mybir.dt.float32
    F32R = mybir.dt.float32r
    BF16 = mybir.dt.bfloat16
    I32 = mybir.dt.int32
    AF = mybir.ActivationFunctionType
    OP = mybir.AluOpType

    nc = bacc.Bacc(None, target_bir_lowering=False)

    def RMM(out, lhsT, rhs, **kw):
        nc.tensor.matmul(out, lhsT=lhsT.bitcast(F32R), rhs=rhs.bitcast(F32R), **kw)

    def P(name, shape, dt=F32):
        return nc.declare_dram_parameter(name, list(shape), dt, isOutput=False)

    # per-core inputs
    embw = P("embw", (EMB_ROWS, NJ))
    embbT = P("embbT", (H, NI))
    x0my = P("x0my", (NI, D))
    maskc = P("maskc", (NI, NJ))
    eyec = P("eyec", (NI, NJ))
    # shared inputs (f32)
    x0rows = P("x0rows", (1, D * NJ))
    c1hiT = P("c1hiT", (L, H, H))
    c1hjT = P("c1hjT", (L, H, H))
    cb1 = P("cb1", (H, L))
    cb2 = P("cb2", (H, L))
    c2T = P("c2T", (L, H, H))
    c3w = P("c3w", (L, H, 2 * NI - 1))
    cb3c = P("cb3c", (NI, L))
    e1hiT = P("e1hiT", (L - 1, H, H))
    eb1 = P("eb1", (H, L - 1))
    eb2 = P("eb2", (H, L - 1))
    nw1hT = P("nw1hT", (L - 1, H, H))
    nw1mT = P("nw1mT", (L - 1, H, H))
    nb1 = P("nb1", (H, L - 1))
    nw2T = P("nw2T", (L - 1, H, H))
    nb2 = P("nb2", (H, L - 1))
    cdrep = P("cdrep", (L, 16 * H))
    edrep = P("edrep", (L - 1, 16 * H))
    ones6k = P("ones6k", (1, 16 * NJ))
    # shared inputs (bf16, edge pass)
    e1hjT_b = P("e1hjT_b", (L - 1, H, H), BF16)
    e2T_b = P("e2T_b", (L - 1, H, H), BF16)
    attw_b = P("attw_b", (L - 1, H, 2 * NI - 1), BF16)

    o_x = nc.declare_dram_parameter("o_x", [N, D], F32, isOutput=True)

    # DRAM internals
    gdram = [nc.dram_tensor(f"gdram{l}", [NI * NJ], BF16) for l in range(L - 1)]
    hag_in = nc.dram_tensor("hag_in", [H, NI], F32)
    hag_out = nc.dram_tensor("hag_out", [NC * H, NI], F32, addr_space="Shared")
    xag_in = [nc.dram_tensor(f"xag_in{l}", [D, NI], F32) for l in range(L - 1)]
    xag_in.append(nc.dram_tensor("xag_in3", [NI, D], F32))
    xag_out = [nc.dram_tensor(f"xag_out{l}", [NC, D, NI], F32, addr_space="Shared")
               for l in range(L - 1)]
    xag_out.append(nc.dram_tensor("xag_out3", [N, D], F32, addr_space="Shared"))
    bar_in = nc.dram_tensor("bar_in", [1, 1], F32)
    bar_out = nc.dram_tensor("bar_out", [NC, 1], F32, addr_space="Shared")
    mag_in = [nc.dram_tensor(f"mag_in{l}", [H, NI], F32) for l in range(L - 1)]
    mag_out = [nc.dram_tensor(f"mag_out{l}", [NC * H, NI], F32, addr_space="Shared")
               for l in range(L - 1)]
    rg = [list(range(NC))]

    with tile.TileContext(nc) as tc:
        with (
            tc.tile_pool(name="consts", bufs=1) as consts,
            tc.tile_pool(name="embp", bufs=3) as embp,
            tc.tile_pool(name="cp1", bufs=1) as cp1,
            tc.tile_pool(name="cp2", bufs=2) as cp2,
            tc.tile_pool(name="work", bufs=2) as work,
            tc.tile_pool(name="slab", bufs=1) as slabp,
            tc.tile_pool(name="gbp", bufs=6) as gbp,
            tc.tile_pool(name="ps_mlp", bufs=2, space="PSUM") as ps_mlp,
            tc.tile_pool(name="ps_acc", bufs=1, space="PSUM") as ps_acc,
            tc.tile_pool(name="ps_nd", bufs=1, space="PSUM") as ps_nd,
        ):
            # ---- phase 0 ----
            # A tiny leading AllGather absorbs one-time cross-core launch
            # skew while the embedding DMAs stream on sync/scalar queues.
            nc.gpsimd.collective_compute(
                "AllGather", OP.bypass, replica_groups=rg,
                ins=[bar_in[:]], outs=[bar_out[:]])
            emb_tiles = []
            qrot = [nc.sync, nc.scalar]
            for t in range(NEMB):
                et = embp.tile([H, EB, NJ], F32, tag="embt")
                qrot[t % 2].dma_start(
                    out=et,
                    in_=embw[t * EB * H:(t + 1) * EB * H, :]
                    .rearrange("(a p) j -> p a j", p=H))
                emb_tiles.append(et)

            # ---- constants (scalar queue; ACT is idle during emb) ----
            def load(pname, ap_in, shape, dt=F32, rnd=False):
                t = consts.tile(list(shape), dt, tag=pname)
                o = t[:].bitcast(F32R) if rnd else t
                nc.scalar.dma_start(
                    out=o, in_=ap_in.bitcast(F32R) if rnd else ap_in)
                return t

            x0my_sb = load("x0my", x0my[:], (NI, D))
            embbT_sb = load("embbT", embbT[:], (H, NI))
            maskc_sb = load("maskc", maskc[:], (NI, NJ))
            eyec_sb = load("eyec", eyec[:], (NI, NJ))
            c1hiT_sb = load("c1hiT", c1hiT.rearrange("l p x -> p l x"), (H, L, H),
                            rnd=True)
            c1hjT_sb = load("c1hjT", c1hjT.rearrange("l p x -> p l x"), (H, L, H),
                            rnd=True)
            c2T_sb = load("c2T", c2T.rearrange("l p x -> p l x"), (H, L, H),
                          rnd=True)
            c3w_sb = load("c3w", c3w.rearrange("l p x -> p l x"),
                          (H, L, 2 * NI - 1), rnd=True)
            cb1_sb = load("cb1", cb1[:], (H, L))
            cb2_sb = load("cb2", cb2[:], (H, L))
            cb3c_sb = load("cb3c", cb3c[:], (NI, L))
            e1hiT_sb = load("e1hiT", e1hiT.rearrange("l p x -> p l x"),
                            (H, L - 1, H), rnd=True)
            eb1_sb = load("eb1", eb1[:], (H, L - 1))
            eb2_sb = load("eb2", eb2[:], (H, L - 1))
            nw1hT_sb = load("nw1hT", nw1hT.rearrange("l p x -> p l x"),
                            (H, L - 1, H), rnd=True)
            nw1mT_sb = load("nw1mT", nw1mT.rearrange("l p x -> p l x"),
                            (H, L - 1, H), rnd=True)
            nb1_sb = load("nb1", nb1[:], (H, L - 1))
            nw2T_sb = load("nw2T", nw2T.rearrange("l p x -> p l x"),
                           (H, L - 1, H), rnd=True)
            nb2_sb = load("nb2", nb2[:], (H, L - 1))
            cdrep_sb = load("cdrep", cdrep[:], (L, 16 * H), rnd=True)
            edrep_sb = load("edrep", edrep[:], (L - 1, 16 * H), rnd=True)
            e1hjT_w = load("e1hjT_b", e1hjT_b.rearrange("l p x -> p l x"),
                           (H, L - 1, H), BF16)
            e2T_w = load("e2T_b", e2T_b.rearrange("l p x -> p l x"),
                         (H, L - 1, H), BF16)
            attw_w = load("attw_b", attw_b.rearrange("l p x -> p l x"),
                          (H, L - 1, 2 * NI - 1), BF16)

            # d2/ones interleaved stripes (f32r): rows 32k = d2 rows
            # 16k..16k+15 flattened, rows 32k+1 = ones.
            d2ones = consts.tile([66, 16 * NJ], F32, tag="d2ones")
            combC = consts.tile([66, 16 * H], F32, tag="combC")
            combE = consts.tile([66, 16 * H], F32, tag="combE")
            for k in range(3):
                nc.scalar.dma_start(
                    out=d2ones[32 * k + 1:32 * k + 2, :].bitcast(F32R),
                    in_=ones6k[:].bitcast(F32R))

            def coord_prep(l, x_my):
                """xb loads + diff/d2/u chain + d2 stripes for layer l."""
                diff = []
                for c in range(D):
                    xb = cp1.tile([NI, NJ], F32, tag=f"xb{c}")
                    if l == 0:
                        bsrc = bass.AP(tensor=x0rows, offset=c * NJ,
                                       ap=[[0, NI], [1, NJ]])
                    else:
                        bsrc = bass.AP(tensor=xag_out[l - 1], offset=c * NI,
                                       ap=[[0, NI], [D * NI, NC], [1, NI]])
                    nc.sync.dma_start(out=xb, in_=bsrc)
                    dc = cp2.tile([NI, NJ], F32, tag=f"diff{c}")
                    nc.vector.tensor_scalar(
                        out=dc, in0=xb, scalar1=x_my[:, c:c + 1], scalar2=None,
                        op0=OP.subtract)
                    diff.append(dc)
                d2 = cp1.tile([NI, NJ], F32, tag="d2")
                tmp = cp1.tile([NI, NJ], F32, tag="ctmp")
                nc.vector.tensor_tensor(out=d2, in0=diff[0], in1=diff[0],
                                        op=OP.mult)
                nc.vector.tensor_tensor(out=tmp, in0=diff[1], in1=diff[1],
                                        op=OP.mult)
                nc.vector.tensor_tensor(out=d2, in0=d2, in1=tmp, op=OP.add)
                nc.vector.tensor_tensor(out=tmp, in0=diff[2], in1=diff[2],
                                        op=OP.mult)
                nc.vector.tensor_tensor(out=d2, in0=d2, in1=tmp, op=OP.add)
                for k in range(3):
                    nc.sync.dma_start(
                        out=d2ones[32 * k:32 * k + 1, :].bitcast(F32R),
                        in_=d2[16 * k:16 * (k + 1), :].bitcast(F32R))
                def emit_u():
                    d2s = cp1.tile([NI, NJ], F32, tag="d2s")
                    tmq = cp1.tile([NI, NJ], F32, tag="tmq")
                    nc.vector.tensor_tensor(out=d2s, in0=d2, in1=eyec_sb,
                                            op=OP.add)
                    # sqrt via bit-trick seed + 1 Newton step (approx recip)
                    sq = cp1.tile([NI, NJ], F32, tag="sq")
                    with nc.allow_low_precision(reason="bit-trick sqrt"):
                        nc.vector.tensor_scalar(
                            out=sq.bitcast(I32), in0=d2s.bitcast(I32),
                            scalar1=1, scalar2=None,
                            op0=OP.logical_shift_right)
                        nc.vector.tensor_scalar(
                            out=sq.bitcast(I32), in0=sq.bitcast(I32),
                            scalar1=QUAKE, scalar2=None, op0=OP.add)
                    nc.vector.reciprocal_approx_fast(out=tmq, in_=sq)
                    nc.vector.tensor_tensor(out=tmq, in0=d2s, in1=tmq,
                                            op=OP.mult)
                    nc.vector.tensor_tensor(out=sq, in0=sq, in1=tmq, op=OP.add)
                    nc.vector.tensor_scalar(out=sq, in0=sq, scalar1=0.5,
                                            scalar2=None, op0=OP.mult)
                    nc.vector.tensor_scalar(out=sq, in0=sq, scalar1=1.0,
                                            scalar2=None, op0=OP.add)
                    u = cp2.tile([NI, NJ], F32, tag="u")
                    nc.vector.reciprocal_approx_fast(out=u, in_=sq)
                    nc.vector.tensor_tensor(out=u, in0=u, in1=maskc_sb,
                                            op=OP.mult)
                    return u
                return diff, emit_u

            diff0, emit_u0 = coord_prep(0, x0my_sb)
            prep0 = (diff0, emit_u0())

            # ---- embedding reduce + h AllGather ----
            hT0 = cp1.tile([H, NI], F32, tag="hT0")
            for t in range(NEMB):
                with nc.allow_low_precision(reason="f32r storage, f32 accum"):
                    nc.vector.tensor_reduce(
                        out=hT0[:, t * EB:(t + 1) * EB].bitcast(F32R),
                        in_=emb_tiles[t],
                        axis=mybir.AxisListType.X, op=OP.add)
            nc.vector.tensor_tensor(out=hT0[:].bitcast(F32R), in0=hT0,
                                    in1=embbT_sb, op=OP.add)
            nc.gpsimd.dma_start(out=hag_in[:], in_=hT0)
            nc.gpsimd.collective_compute(
                "AllGather", OP.bypass, replica_groups=rg,
                ins=[hag_in[:]], outs=[hag_out[:]])
            h_T = cp2.tile([H, NJ], F32, tag="hT")
            nc.scalar.dma_start(
                out=h_T[:].bitcast(F32R).rearrange("p (r n) -> p r n", r=NC),
                in_=hag_out.rearrange("(r p) n -> p r n", p=H).bitcast(F32R))
            h_my = hT0
            hT_bf = cp2.tile([H, NJ], BF16, tag="hTb")
            nc.vector.tensor_copy(hT_bf, h_T)

            x_my = x0my_sb

            for l in range(L):
                if KTRUNC == 1 or (KTRUNC and l > 0):
                    break
                last = l == L - 1
                do_gate = (not last) and KTRUNC in (0, 3, 4)
                do_coord = KTRUNC in (0, 3, 4)
                do_node = (not last) and KTRUNC in (0, 4)

                if l == 0:
                    diff, u = prep0
                    emit_u = None
                else:
                    diff, emit_u = coord_prep(l, x_my)
                    u = None

                # A-terms: At[i, :] = (W1hi @ h_i), striped into comb pairs
                def make_comb(hiT, wdrep_row, comb, atag):
                    At_ps = ps_nd.tile([NI, H], F32, tag="nd")
                    RMM(At_ps, lhsT=h_my, rhs=hiT, start=True, stop=True)
                    At = cp1.tile([NI, H], F32, tag=atag)
                    nc.vector.tensor_copy(At[:].bitcast(F32R), At_ps)
                    for k in range(3):
                        nc.scalar.dma_start(
                            out=comb[32 * k:32 * k + 1, :].bitcast(F32R),
                            in_=wdrep_row.bitcast(F32R))
                        nc.scalar.dma_start(
                            out=comb[32 * k + 1:32 * k + 2, :].bitcast(F32R),
                            in_=At[16 * k:16 * (k + 1), :].bitcast(F32R))

                make_comb(c1hiT_sb[:, l, :], cdrep_sb[l:l + 1, :], combC, "Atc")
                if not last:
                    make_comb(e1hiT_sb[:, l, :], edrep_sb[l:l + 1, :], combE,
                              "Ate")

                # group g covers rows {g, 16+g, 32+g}: the three rank-2
                # matmuls hit PE row-quadrants 0/32/64 and run concurrently
                def mm_d(pre, comb, g):
                    for r in range(G):
                        nc.tensor.matmul(
                            pre[:, r * 512:r * 512 + NJ],
                            lhsT=comb[32 * r:32 * r + 2,
                                      g * H:(g + 1) * H].bitcast(F32R),
                            rhs=d2ones[32 * r:32 * r + 2,
                                       g * NJ:(g + 1) * NJ].bitcast(F32R),
                            start=False, stop=True)

                # ---- edge pass (bf16) ----
                if not last:
                    att_ps = ps_acc.tile([H, NJ], F32, tag="acc")
                    m2slab = slabp.tile([H, NI * NJ], BF16, tag="m2")

                    def edge_tail(item):
                        t1, g = item
                        z2 = ps_mlp.tile([H, G * 512], F32, tag="mlp")
                        nc.tensor.matmul(z2[:, 0:512], lhsT=e2T_w[:, l, :],
                                         rhs=t1[:, 0:512], start=True,
                                         stop=True)
                        nc.tensor.matmul(z2[:, 512:1024], lhsT=e2T_w[:, l, :],
                                         rhs=t1[:, 512:1024], start=True,
                                         stop=True)
                        nc.tensor.matmul(z2[:, 1024:1152], lhsT=e2T_w[:, l, :],
                                         rhs=t1[:, 1024:1152], start=True,
                                         stop=True)
                        nc.scalar.activation(
                            out=m2slab[:, g * G * NJ:(g + 1) * G * NJ],
                            in_=z2[:, 0:G * NJ],
                            func=AF.Silu, bias=eb2_sb[:, l:l + 1], scale=1.0)
                        for r in range(G):
                            i = 16 * r + g
                            s = G * g + r
                            nc.tensor.matmul(
                                att_ps[0:NI, :],
                                lhsT=attw_w[:, l, (NI - 1) - i:(2 * NI - 1) - i],
                                rhs=m2slab[:, s * NJ:(s + 1) * NJ],
                                start=(s == 0), stop=(s == NI - 1))

                    epend = None
                    for g in range(NGRP):
                        pre = ps_mlp.tile([H, G * 512], F32, tag="mlp")
                        for r in range(G):
                            nc.tensor.matmul(
                                pre[:, r * 512:r * 512 + NJ],
                                lhsT=e1hjT_w[:, l, :], rhs=hT_bf,
                                start=True, stop=False)
                        mm_d(pre, combE, g)
                        t1 = work.tile([H, G * NJ], BF16, tag="t1")
                        nc.scalar.activation(
                            out=t1.rearrange("p (r c) -> p r c", r=G),
                            in_=pre.rearrange("p (r c) -> p r c", r=G)[:, :, 0:NJ],
                            func=AF.Silu, bias=eb1_sb[:, l:l + 1], scale=1.0)
                        if epend is not None:
                            edge_tail(epend)
                        epend = (t1, g)
                    edge_tail(epend)
                    if emit_u is not None:
                        u = emit_u()
                        emit_u = None

                    # gate: exact sigmoid via tanh (same ACT table as silu)
                    if not do_gate:
                        continue
                    sg = cp1.tile([NI, NJ], F32, tag="sg")
                    nc.scalar.activation(out=sg, in_=att_ps[0:NI, :],
                                         func=AF.Tanh, bias=0.0, scale=0.5)
                    gmask = cp1.tile([NI, NJ], F32, tag="gmask")
                    nc.vector.tensor_scalar(out=gmask, in0=sg, scalar1=0.5,
                                            scalar2=0.5, op0=OP.mult,
                                            op1=OP.add)
                    nc.vector.tensor_tensor(out=gmask, in0=gmask, in1=maskc_sb,
                                            op=OP.mult)
                    gmb = cp1.tile([NI, NJ], BF16, tag="gmb")
                    nc.vector.tensor_copy(gmb, gmask)
                    nc.sync.dma_start(out=gdram[l][:], in_=gmb)
                    # partition-broadcast gate rows via DRAM stride-0 reads,
                    # in slab-slot order s = G*g + r  ->  grid row 16*r + g
                    gb_tiles = [None] * NI
                    for g in range(NGRP):
                        for r in range(G):
                            i = 16 * r + g
                            s = G * g + r
                            gb = gbp.tile([H, NJ], BF16, tag="gb")
                            eng = nc.sync if s % 2 == 0 else nc.gpsimd
                            eng.dma_start(
                                out=gb,
                                in_=bass.AP(tensor=gdram[l], offset=i * NJ,
                                            ap=[[0, H], [1, NJ]]))
                            gb_tiles[s] = gb
                    msumT = cp1.tile([H, NI], F32, tag="msumT")
                    mgs = cp1.tile([H, NJ], BF16, tag="mgs")

                # ---- coord pass (f32r) ----
                if do_coord:
                    if emit_u is not None:
                        u = emit_u()
                        emit_u = None
                    phi_ps = ps_acc.tile([H, NJ], F32, tag="acc")

                    def msum_row(s):
                        i = 16 * (s % G) + s // G
                        nc.vector.scalar_tensor_tensor(
                            out=mgs, in0=m2slab[:, s * NJ:(s + 1) * NJ],
                            scalar=1.0, in1=gb_tiles[s],
                            op0=OP.mult, op1=OP.mult,
                            accum_out=msumT[:, i:i + 1])

                    def coord_tail(item):
                        t1c, g = item
                        z2 = ps_mlp.tile([H, G * 512], F32, tag="mlp")
                        RMM(z2[:, 0:512], lhsT=c2T_sb[:, l, :],
                            rhs=t1c[:, 0:512], start=True, stop=True)
                        RMM(z2[:, 512:1024], lhsT=c2T_sb[:, l, :],
                            rhs=t1c[:, 512:1024], start=True, stop=True)
                        RMM(z2[:, 1024:1152], lhsT=c2T_sb[:, l, :],
                            rhs=t1c[:, 1024:1152], start=True, stop=True)
                        t2c = work.tile([H, G * NJ], F32, tag="t2c")
                        nc.scalar.activation(
                            out=t2c[:].bitcast(F32R),
                            in_=z2[:, 0:G * NJ],
                            func=AF.Silu, bias=cb2_sb[:, l:l + 1], scale=1.0)
                        for r in range(G):
                            i = 16 * r + g
                            s = G * g + r
                            RMM(phi_ps[0:NI, :],
                                lhsT=c3w_sb[:, l,
                                            (NI - 1) - i:(2 * NI - 1) - i],
                                rhs=t2c[:, r * NJ:(r + 1) * NJ],
                                start=(s == 0), stop=(s == NI - 1))
                        if do_gate:
                            for r in range(G):
                                msum_row(G * g + r)

                    cpend = None
                    for g in range(NGRP):
                        pre = ps_mlp.tile([H, G * 512], F32, tag="mlp")
                        for r in range(G):
                            RMM(pre[:, r * 512:r * 512 + NJ],
                                lhsT=c1hjT_sb[:, l, :], rhs=h_T,
                                start=True, stop=False)
                        mm_d(pre, combC, g)
                        t1c = work.tile([H, G * NJ], F32, tag="t1c")
                        nc.scalar.activation(
                            out=t1c[:].bitcast(F32R)
                            .rearrange("p (r c) -> p r c", r=G),
                            in_=pre.rearrange("p (r c) -> p r c", r=G)[:, :, 0:NJ],
                            func=AF.Silu, bias=cb1_sb[:, l:l + 1], scale=1.0)
                        if cpend is not None:
                            coord_tail(cpend)
                        cpend = (t1c, g)
                    coord_tail(cpend)

                if do_node:
                    # msum AllGather + node MLP (overlaps late coord groups)
                    nc.gpsimd.dma_start(out=mag_in[l][:], in_=msumT)
                    nc.gpsimd.collective_compute(
                        "AllGather", OP.bypass, replica_groups=rg,
                        ins=[mag_in[l][:]], outs=[mag_out[l][:]])
                    msumF = cp1.tile([H, NJ], F32, tag="msumF")
                    nc.gpsimd.dma_start(
                        out=msumF[:].bitcast(F32R)
                        .rearrange("p (r n) -> p r n", r=NC),
                        in_=mag_out[l].rearrange("(r p) n -> p r n", p=H)
                        .bitcast(F32R))
                    z1 = ps_nd.tile([H, 512], F32, tag="nd")
                    RMM(z1[:, 0:NJ], lhsT=nw1hT_sb[:, l, :], rhs=h_T,
                        start=True, stop=False)
                    RMM(z1[:, 0:NJ], lhsT=nw1mT_sb[:, l, :], rhs=msumF,
                        start=False, stop=True)
                    t1n = cp1.tile([H, NJ], F32, tag="t1n")
                    nc.scalar.activation(out=t1n[:].bitcast(F32R),
                                         in_=z1[:, 0:NJ], func=AF.Silu,
                                         bias=nb1_sb[:, l:l + 1], scale=1.0)
                    z2n = ps_nd.tile([H, 512], F32, tag="nd")
                    RMM(z2n[:, 0:NJ], lhsT=nw2T_sb[:, l, :], rhs=t1n,
                        start=True, stop=True)
                    h_T = cp2.tile([H, NJ], F32, tag="hT")
                    nc.vector.tensor_scalar(out=h_T[:].bitcast(F32R),
                                            in0=z2n[:, 0:NJ],
                                            scalar1=nb2_sb[:, l:l + 1],
                                            scalar2=None, op0=OP.add)
                    if l < L - 2:
                        hT_bf = cp2.tile([H, NJ], BF16, tag="hTb")
                        nc.vector.tensor_copy(hT_bf, h_T)
                    # local copy of this core's own h rows
                    z1m = ps_nd.tile([H, 512], F32, tag="nd")
                    nc.tensor.matmul(z1m[:, 0:NI], lhsT=nw1hT_sb[:, l, :],
                                     rhs=h_my, start=True, stop=False)
                    nc.tensor.matmul(z1m[:, 0:NI], lhsT=nw1mT_sb[:, l, :],
                                     rhs=msumT, start=False, stop=True)
                    t1m = cp1.tile([H, NI], F32, tag="t1m")
                    nc.scalar.activation(out=t1m, in_=z1m[:, 0:NI],
                                         func=AF.Silu,
                                         bias=nb1_sb[:, l:l + 1], scale=1.0)
                    z2m = ps_nd.tile([H, 512], F32, tag="nd")
                    nc.tensor.matmul(z2m[:, 0:NI], lhsT=nw2T_sb[:, l, :],
                                     rhs=t1m, start=True, stop=True)
                    h_my = cp2.tile([H, NI], F32, tag="hmy")
                    nc.vector.tensor_scalar(out=h_my[:].bitcast(F32R),
                                            in0=z2m[:, 0:NI],
                                            scalar1=nb2_sb[:, l:l + 1],
                                            scalar2=None, op0=OP.add)

                if do_coord:
                    # ---- phi stream + x update ----
                    phis = cp1.tile([NI, NJ], F32, tag="phis")
                    nc.vector.tensor_scalar(out=phis, in0=phi_ps[0:NI, :],
                                            scalar1=cb3c_sb[:, l:l + 1],
                                            scalar2=None, op0=OP.add)
                    s = cp1.tile([NI, NJ], F32, tag="s")
                    nc.vector.tensor_tensor(out=s, in0=phis, in1=u, op=OP.mult)
                    xnew = cp2.tile([NI, D], F32, tag="xnew")
                    xms = cp1.tile([NI, NJ], F32, tag="xms")
                    for c in range(D):
                        xcol = cp1.tile([NI, 1], F32, tag=f"xcol{c}")
                        nc.vector.scalar_tensor_tensor(
                            out=xms, in0=diff[c], scalar=1.0, in1=s,
                            op0=OP.mult, op1=OP.mult, accum_out=xcol)
                        nc.vector.tensor_tensor(
                            out=xnew[:, c:c + 1], in0=xcol,
                            in1=x_my[:, c:c + 1], op=OP.add)
                    if not last:
                        nc.gpsimd.dma_start(
                            out=xag_in[l].rearrange("c n -> n c"), in_=xnew)
                    else:
                        nc.gpsimd.dma_start(out=xag_in[l][:], in_=xnew)
                    nc.gpsimd.collective_compute(
                        "AllGather", OP.bypass, replica_groups=rg,
                        ins=[xag_in[l][:]], outs=[xag_out[l][:]])
                    if not last:
                        x_my = xnew
                    else:
                        nc.sync.dma_start(out=o_x[:], in_=xag_out[l][:])

            if KTRUNC:
                nc.sync.dma_start(out=o_x[0:NI, :], in_=x0my_sb)

    nc.finalize()
    return nc


def _prep_inputs(inputs):
    import ml_dtypes
    BF = ml_dtypes.bfloat16
    f = lambda a: np.ascontiguousarray(np.asarray(a), dtype=np.float32)
    b = lambda a: np.ascontiguousarray(np.asarray(a, dtype=np.float32)
                                       .astype(BF))
    x_inp = f(inputs["x_inp"])
    emb_w = f(inputs["emb_w"])
    emb_b = f(inputs["emb_b"])
    coord_w1 = f(inputs["coord_w1"])
    coord_b1 = f(inputs["coord_b1"])
    coord_w2 = f(inputs["coord_w2"])
    coord_b2 = f(inputs["coord_b2"])
    coord_w3 = f(inputs["coord_w3"])
    coord_b3 = f(inputs["coord_b3"])
    edge_w1 = f(inputs["edge_w1"])
    edge_b1 = f(inputs["edge_b1"])
    edge_w2 = f(inputs["edge_w2"])
    edge_b2 = f(inputs["edge_b2"])
    node_w1 = f(inputs["node_w1"])
    node_b1 = f(inputs["node_b1"])
    node_w2 = f(inputs["node_w2"])
    node_b2 = f(inputs["node_b2"])
    att_w = f(inputs["att_w"])

    x0 = x_inp.reshape(N, D)
    eye = np.eye(N, dtype=np.float32)

    def stackT(w, lo, hi):
        return np.ascontiguousarray(
            np.stack([w[l, :, lo:hi].T for l in range(w.shape[0])]))

    def win(w3):
        nl = w3.shape[0]
        out = np.zeros((nl, H, 2 * NI - 1), np.float32)
        out[:, :, NI - 1] = w3[:, 0, :]
        return out

    shared = dict(
        x0rows=np.ascontiguousarray(x0.T.reshape(1, D * N)),
        c1hiT=stackT(coord_w1, 0, H),
        c1hjT=stackT(coord_w1, H, 2 * H),
        cb1=np.ascontiguousarray(coord_b1.T),
        cb2=np.ascontiguousarray(coord_b2.T),
        c2T=np.ascontiguousarray(np.stack([coord_w2[l].T for l in range(L)])),
        c3w=win(coord_w3),
        cb3c=np.ascontiguousarray(
            np.broadcast_to(coord_b3[:, 0][None, :], (NI, L))),
        e1hiT=stackT(edge_w1, 0, H),
        eb1=np.ascontiguousarray(edge_b1.T),
        eb2=np.ascontiguousarray(edge_b2.T),
        nw1hT=stackT(node_w1, 0, H),
        nw1mT=stackT(node_w1, H, 2 * H),
        nb1=np.ascontiguousarray(node_b1.T),
        nw2T=np.ascontiguousarray(np.stack([node_w2[l].T for l in range(L - 1)])),
        nb2=np.ascontiguousarray(node_b2.T),
        cdrep=np.ascontiguousarray(np.tile(coord_w1[:, :, 2 * H], (1, 16))),
        edrep=np.ascontiguousarray(np.tile(edge_w1[:, :, 2 * H], (1, 16))),
        ones6k=np.ones((1, 16 * NJ), np.float32),
        e1hjT_b=b(stackT(edge_w1, H, 2 * H)),
        e2T_b=b(np.stack([edge_w2[l].T for l in range(L - 1)])),
        attw_b=b(win(att_w)),
    )
    in_maps = []
    for c in range(NC):
        m = dict(shared)
        m["embw"] = np.ascontiguousarray(
            emb_w[c * EMB_ROWS:(c + 1) * EMB_ROWS, :])
        m["embbT"] = np.ascontiguousarray(
            emb_b[c * EMB_ROWS:(c + 1) * EMB_ROWS].reshape(NI, H).T)
        m["x0my"] = np.ascontiguousarray(x0[c * NI:(c + 1) * NI, :])
        m["maskc"] = np.ascontiguousarray(1.0 - eye[c * NI:(c + 1) * NI, :])
        m["eyec"] = np.ascontiguousarray(eye[c * NI:(c + 1) * NI, :])
        in_maps.append(m)
    return in_maps


def _run(inputs, trace=False, **kw):
    from concourse.bass_utils import run_bass_kernel_spmd
    if "nc" not in _cache:
        _cache["nc"] = _build_nc()
    in_maps = _prep_inputs(inputs)
    return run_bass_kernel_spmd(_cache["nc"], in_maps, list(range(NC)),
                                trace=trace, **kw)


def kernel(**inputs) -> np.ndarray:
    res = _run(inputs)
    return np.asarray(res.results[0]["o_x"], dtype=np.float32).reshape(N * D)
